# revision 33
# baseline (speedup 1.0000x reference)
"""Causal self-attention (B=2, T=2048, C=1024, H=16) on 8 TRN2 NeuronCores.

Sharding: core c handles batch b = c//4 and heads 4*(c%4) .. 4*(c%4)+3
(data-parallel over B, tensor-parallel over heads; full K/V for its heads
is computed locally from the core's QKV projection slice).

Optimizations over the f32r baseline (185us -> ~126us):
  - all matmul operands bf16 (separate LDWEIGHTS with FWL + pull-ahead
    instead of f32r's self-loading weight path; no f32r N<256 4x penalty;
    half the DMA bytes); PSUM accumulation stays f32
  - the two K=64 S-matmuls of a head pair land in disjoint PE row groups
    (auto tile_position from base partitions 0/64) and execute
    concurrently (second MM ends ~3ns after the first)
  - exp only over causally-valid columns; causal mask shrunk to the one
    128-column staircase band per diagonal tile (PV skips dead columns,
    so only the band needs zeroing) -> ~4x less gpsimd and a shorter
    exp->mask->PV critical path
  - softmax division done on the host during unshard (row 64 of each
    head's output carries the denominators); kills the on-device
    reciprocal/broadcast/multiply epilogue and its serial tail
  - startup was DMA descriptor-gen bound (~0.65us per dma_start,
    serialized per HWDGE ring): weights arrive pre-tiled so all input
    lands in 9 dma_starts issued in strict need-order, and the HAM
    warm-up bridges until the first chain's data arrives
  - projection chains are spliced between attention chunks in 4-matmul
    units so the in-order PE always has ready work while ScalarE grinds
    exp (the attention steady state is exp-bound at ~1.0us/chunk)

Per-core dataflow:
  - host passes xT = x[b].T [C,T] bf16, wqk [128, ft*1024+c*128+f] bf16
    (ft-major pre-tiled q/k weights), wv [128, c*260+f] bf16 (per head a
    65-wide block whose last column is zero)
  - qT/kT [64,T] per head via projection matmuls (contraction c on
    partitions), PSUM f32, cast to bf16 on the PSUM->SBUF copy
  - v [t,260] bf16 with a ones column appended per head (65th of a block)
  - head pairs (2p, 2p+1) share S^T tiles: s [k=128, 1024] = [S_even|S_odd]
    f32 PSUM, exp on ScalarE (scale=1/8 fused) -> bf16 pt
  - y^T [65, 512] += V'.T @ P^T accumulated over k-tiles; row 64 = softmax
    denominators (from the ones column)
  - y^T copied PSUM->SBUF bf16 (vector/scalar alternating), DMA'd out;
    host divides rows 0:64 by row 64, transposes, concats heads.
"""

import os
import sys
import types
import numpy as np
import ml_dtypes

import concourse.bass as bass
import concourse.mybir as mybir
import concourse.tile as tile
from concourse import bacc
from concourse.bass_utils import run_bass_kernel_spmd

B, T, C, H = 2, 2048, 1024, 16
D = 64
NCORES = 8
HPC = 4          # heads per core
NQB = 4          # q blocks of 512
QB = 512
F32 = mybir.dt.float32
BF16 = mybir.dt.bfloat16
EXP = mybir.ActivationFunctionType.Exp
IS_GE = mybir.AluOpType.is_ge


def _install_profhook():
    """Register the NTFF profile hook shim so BASS_TRACE=1 works; harmless
    no-op (graceful trace skip) when the axon .so lacks profiling."""
    if "antenv.axon_hooks" not in sys.modules:
        mod = types.ModuleType("antenv.axon_hooks")
        mod._hook = None
        mod.set_axon_ntff_profile_hook = lambda h: setattr(mod, "_hook", h)
        mod.get_axon_ntff_profile_hook = lambda: mod._hook
        sys.modules["antenv.axon_hooks"] = mod
        try:
            import antenv
            antenv.axon_hooks = mod
        except ImportError:
            pass
    try:
        from trn_agent_boot.trn_boot import _ntff_profile_via_ctypes
        sys.modules["antenv.axon_hooks"].set_axon_ntff_profile_hook(
            _ntff_profile_via_ctypes("/opt/axon/libaxon_pjrt.so")
        )
        import concourse.bass_utils as bu
        bu.upload_artifacts = lambda tmpdir: tmpdir
    except Exception:
        pass


_install_profhook()

_NC = None


def _build():
    nc = bacc.Bacc("TRN2", target_bir_lowering=False, debug=False,
                   num_devices=NCORES)
    # weights arrive pre-tiled on the partition axis (row p holds c-tile
    # slice [c*128+p, :] at columns c*W..) so each loads as ONE dma_start:
    # descriptor generation is ~0.65us per dma_start regardless of size,
    # and the startup was descriptor-gen bound
    xT_d = nc.declare_dram_parameter("xT", [C, T], BF16, isOutput=False)
    wqk_d = nc.declare_dram_parameter("wqk", [128, 4 * 8 * 128], BF16,
                                      isOutput=False)
    wv_d = nc.declare_dram_parameter("wv", [128, 8 * 260], BF16,
                                     isOutput=False)
    y_d = nc.declare_dram_parameter("y", [HPC, 65, T], BF16, isOutput=True)

    from contextlib import ExitStack
    with tile.TileContext(nc) as tc, ExitStack() as ctx:
        sb = ctx.enter_context(tc.tile_pool(name="sb", bufs=1))
        pp = ctx.enter_context(tc.tile_pool(name="pp", bufs=8))
        yp = ctx.enter_context(tc.tile_pool(name="yp", bufs=3))
        psp = ctx.enter_context(tc.tile_pool(name="psp", bufs=2, space="PSUM"))
        pss = ctx.enter_context(tc.tile_pool(name="pss", bufs=2, space="PSUM"))
        psy = ctx.enter_context(tc.tile_pool(name="psy", bufs=1, space="PSUM"))

        # x^T merged per t-block: xall[tb][p, c*512+f] = x[b].T[c*128+p,
        # tb*512+f] — each t-block loads as ONE dma_start (startup is both
        # descriptor-gen and bandwidth bound, so fewest gens in strict
        # need-order wins). wqk is ft-major so the first q-chain's weights
        # are their own small transfer.
        xall = [sb.tile([128, 8 * 512], BF16, name=f"xall{tb}")
                for tb in range(4)]
        wqks_all = sb.tile([128, 4 * 8 * 128], BF16, name="wqks")
        wvs_all = sb.tile([128, 8 * 260], BF16, name="wvs")
        qs = [[sb.tile([128, 512], BF16, name=f"q{p}_{tb}") for tb in range(4)]
              for p in range(2)]
        ks = [[sb.tile([128, 512], BF16, name=f"k{p}_{tb}") for tb in range(4)]
              for p in range(2)]
        vs = [sb.tile([128, 260], BF16, name=f"v_{t}") for t in range(16)]
        ones2 = sb.tile([128, 4], F32, name="ones2")
        nc.gpsimd.memset(ones2[:], 1.0)

        def xslice(tb, lo=0, hi=512):
            """AP slice of x^T covering t-block tb, contraction tile c."""
            return lambda c: xall[tb][:, c * 512 + lo:c * 512 + hi]

        # warm-up: keep the PE's HAM activity monitor busy while the input
        # DMAs land, so real matmuls start at 2.4 GHz instead of 1.2 GHz.
        # The operand memset rides the vector engine (gpsimd wakes ~3 us
        # later), and 22 cold matmuls (~4.7 us) bridge until the first
        # projection chain's inputs have arrived.
        wup = sb.tile([128, 256], BF16, name="wup")
        nc.vector.memset(wup[:], 0.5)
        wups = psp.tile([128, 256], F32, name="wups", tag="pmm")
        for _ in range(22):
            nc.tensor.matmul(wups[:], wup[:, 0:128], wup[:], start=True,
                             stop=True)

        # 8 dma_starts in strict need-order: q-pair0 weights (0.25MB), x
        # t-block 0 (1MB), k-pair0 weights, wv, pair-1 qk weights (one
        # strided transfer), then x t-blocks 1..3
        xT3 = xT_d.ap().rearrange("(c p) t -> p c t", c=8)  # [128, 8, 2048]

        def dma_xall(tb, c0=0, c1=8):
            nc.sync.dma_start(
                xall[tb][:, c0 * 512:c1 * 512].rearrange(
                    "p (c f) -> p c f", c=c1 - c0),
                xT3[:, c0:c1, tb * 512:(tb + 1) * 512])

        nc.sync.dma_start(wqks_all[:, 0:1024], wqk_d.ap()[:, 0:1024])
        # t-block 0 in halves so the first chain's c=0..3 matmuls can
        # start while c=4..7 is still in flight
        dma_xall(0, 0, 4)
        dma_xall(0, 4, 8)
        nc.sync.dma_start(wqks_all[:, 2048:3072], wqk_d.ap()[:, 2048:3072])
        nc.sync.dma_start(wvs_all[:], wv_d.ap()[:, :])
        nc.sync.dma_start(
            wqks_all[:].rearrange("p (b x) -> p b x", b=2)[:, :, 1024:2048],
            wqk_d.ap().rearrange("p (b x) -> p b x", b=2)[:, :, 1024:2048])
        for tb in (1, 2, 3):
            dma_xall(tb)

        qk_cache = {}

        def qk_chain_part(p, ft_kind, tb, half):
            """Half of one projection chain (4 accumulating matmuls); the
            second half finishes the accumulation and casts PSUM->SBUF."""
            ft = p if ft_kind == 0 else 2 + p
            key = (p, ft_kind, tb)
            if half == 0:
                qk_cache[key] = psp.tile([128, 512], F32,
                                         name=f"pqk{p}_{ft}_{tb}", tag="pmm")
            mm = qk_cache[key]
            xs = xslice(tb)
            for c in range(4 * half, 4 * half + 4):
                nc.tensor.matmul(mm[:],
                                 wqks_all[:, ft * 1024 + c * 128:
                                          ft * 1024 + (c + 1) * 128],
                                 xs(c),
                                 start=(c == 0), stop=(c == 7))
            if half == 1:
                dst = (qs if ft_kind == 0 else ks)[p][tb]
                nc.vector.tensor_copy(dst[:], mm[:])
                del qk_cache[key]

        def qk_chain(p, ft_kind, tb):
            qk_chain_part(p, ft_kind, tb, 0)
            qk_chain_part(p, ft_kind, tb, 1)

        def v_chain(tt):
            """Combined v projection for one t-tile (all 4 heads, N=260)."""
            tb, sub = tt // 4, tt % 4
            mmv = psp.tile([128, 260], F32, name=f"pv{tt}", tag="pmm")
            xs = xslice(tb, sub * 128, (sub + 1) * 128)
            for c in range(8):
                nc.tensor.matmul(mmv[:], xs(c),
                                 wvs_all[:, c * 260:(c + 1) * 260],
                                 start=(c == 0), stop=(c == 7))
            nc.vector.tensor_copy(vs[tt][:], mmv[:])
            nc.vector.tensor_copy(vs[tt][:, 64:260:65], ones2[:])

        def attn_s_part(p, j, kk, ptiles):
            """S matmuls + exp + causal mask for chunk (p, j, kk).

            Diagonal k-tiles only have valid scores for q >= k, i.e. local
            q >= off = 128*(kk-4j); matmuls and exp skip the dead columns
            (PV skips them too, so they can hold stale garbage), and only
            the 128-wide staircase band [off, off+128) needs masking."""
            off = max(0, 128 * (kk - 4 * j))
            s = pss.tile([128, 1024], F32, name=f"s{p}_{j}_{kk}", tag="s")
            ktb, ksub = kk // 4, (kk % 4) * 128
            nc.tensor.matmul(s[:, off:512],
                             ks[p][ktb][0:64, ksub:ksub + 128],
                             qs[p][j][0:64, off:512],
                             start=True, stop=True)
            nc.tensor.matmul(s[:, 512 + off:1024],
                             ks[p][ktb][64:128, ksub:ksub + 128],
                             qs[p][j][64:128, off:512],
                             start=True, stop=True)
            pt = pp.tile([128, 1024], BF16, name=f"pt{p}_{j}_{kk}", tag="pt")
            if off:
                nc.scalar.activation(
                    pt[:].rearrange("p (b q) -> p b q", b=2)[:, :, off:512],
                    s[:].rearrange("p (b q) -> p b q", b=2)[:, :, off:512],
                    EXP, scale=0.125)
            else:
                nc.scalar.activation(pt[:], s[:], EXP, scale=0.125)
            if kk >= 4 * j:
                # zero P where q < k; only the staircase band straddles the
                # diagonal (cols [off, off+128) of both head halves); the
                # predicate reduces to local_q >= key_partition (base=0)
                band = pt[:].rearrange("p (b q) -> p b q", b=2)[
                    :, :, off:off + 128]
                nc.gpsimd.affine_select(
                    band, band,
                    pattern=[[0, 2], [1, 128]],
                    compare_op=IS_GE, fill=0.0,
                    base=0,
                    channel_multiplier=-1)
            ptiles[(j, kk)] = pt

        def emit_out(p, j, state):
            """Copy the finished y^T accumulators (incl. denominator row 64)
            PSUM->SBUF as bf16 and DMA out in 4 row-chunks (spread across
            DMA queues); host does the divide. The very last output's
            second copy rides ScalarE (idle by then) so the two tail
            copies run in parallel; midstream both stay off ScalarE,
            which is the attention bottleneck (gpsimd cannot read PSUM)."""
            for h01, key in ((0, "ye"), (1, "yo")):
                ysb = yp.tile([65, 512], BF16,
                              name=f"ysb{p}_{j}_{h01}", tag="ysb")
                last = h01 == 1 and p == 1 and j == NQB - 1
                if last:
                    nc.scalar.copy(ysb[:], state[key][:])
                else:
                    nc.vector.tensor_copy(ysb[:], state[key][:])
                # one dma_start per head tile (the HW splits it across all
                # 16 SDMA engines); the final tile rides the scalar HWDGE
                # ring so the two tail DMAs drain in parallel
                eng = nc.scalar if last else nc.sync
                eng.dma_start(
                    y_d.ap()[2 * p + h01, :, j * 512:(j + 1) * 512], ysb[:])

        def attn_pv_part(p, j, kk, state, ptiles):
            """PV-accumulation closures for chunk (p, j, kk): one matmul per
            head half (so the caller can group same-PSUM-bank matmuls), plus
            a trailing output closure on the q-block's last k-tile."""
            nkt = 4 * (j + 1)
            if kk == 0:
                state["ye"] = psy.tile([65, 512], F32,
                                       name=f"ye{p}_{j}", tag="ye")
                state["yo"] = psy.tile([65, 512], F32,
                                       name=f"yo{p}_{j}", tag="yo")
            pt = ptiles.pop((j, kk))
            first, last = (kk == 0), (kk == nkt - 1)
            # skip columns where P is all-zero (above the causal diagonal);
            # their y contribution is zero and PSUM keeps the prior partials
            off = 0 if first else max(0, 128 * (kk - 4 * j))

            def mm_e():
                nc.tensor.matmul(state["ye"][:, off:512],
                                 vs[kk][:, 130 * p:130 * p + 65],
                                 pt[:, off:512],
                                 start=first, stop=last)

            def mm_o():
                nc.tensor.matmul(state["yo"][:, off:512],
                                 vs[kk][:, 130 * p + 65:130 * p + 130],
                                 pt[:, 512 + off:1024],
                                 start=first, stop=last)

            fin = (lambda: emit_out(p, j, state)) if last else None
            return mm_e, mm_o, fin

        ptiles = {}
        states = {}

        def run_pair(p, stage_work, filler=None, flip_from=0):
            """Emit the pair's attention as one flat pipeline in batches of
            two chunks: S/exp of batch b is emitted before PV of batch b-1
            (across q-block boundaries), so the in-order PE never stalls
            behind exp. stage_work (projection chains) is emitted at a
            q-block's first chunk; filler[i] work units are spliced in
            right after chunk i's S matmuls (PE food during exp waits)."""
            seq = [(j, kk) for j in range(NQB) for kk in range(4 * (j + 1))]
            batches = [seq[i:i + 2] for i in range(0, len(seq), 2)]
            filler = filler or {}

            def emit_pv(batch):
                parts = [attn_pv_part(p, pj, pkk,
                                      states.setdefault((p, pj), {}), ptiles)
                         for (pj, pkk) in batch]
                for e, o, _ in parts:
                    e()
                    o()
                for _, _, fin in parts:
                    if fin is not None:
                        fin()

            prev = None
            ci = 0
            for batch in batches:
                for (j, kk) in batch:
                    if kk == 0:
                        for w in stage_work.get(j, ()):
                            w()
                    # filler BEFORE the chunk's S matmuls: when S stalls at
                    # the in-order PE head on the s-tile recycle (exp
                    # pacing), ready projection work must sit ahead of it,
                    # not behind. The first startup chunks keep filler
                    # after-S so the first exps dispatch ASAP.
                    fl = filler.get(ci, ())
                    if ci >= flip_from:
                        for w in fl:
                            w()
                    attn_s_part(p, j, kk, ptiles)
                    if ci < flip_from:
                        for w in fl:
                            w()
                    ci += 1
                if prev is not None:
                    emit_pv(prev)
                prev = batch
            emit_pv(prev)

        # pair-0 stages: its own q/k projections + the first v tile of the
        # block; the remaining v tiles and pair 1's t-block-0 projections
        # are spliced between chunks as filler (q-blocks start at chunk
        # indices 0, 4, 12, 24; v(4j+i) must land ~i chunks in, before the
        # PV that consumes it)
        stage0 = {}
        for j in range(NQB):
            stage0[j] = [lambda j=j: qk_chain(0, 0, j),
                         lambda j=j: qk_chain(0, 1, j)]
            if j:
                stage0[j].append(lambda j=j: v_chain(4 * j))
        fill0 = {0: [lambda: v_chain(0)]}
        for j, base in enumerate((0, 4, 12, 24)):
            for i in (1, 2, 3):
                fill0.setdefault(base + i - 1, []).append(
                    lambda tt=4 * j + i: v_chain(tt))

        def funit(tb, ft_kind, half):
            return lambda: qk_chain_part(1, ft_kind, tb, half)

        for ci, (tb, ft_kind, half) in [
                (27, (0, 0, 0)), (29, (0, 0, 1)),
                (31, (0, 1, 0)), (33, (0, 1, 1))]:
            fill0.setdefault(ci, []).append(funit(tb, ft_kind, half))

        # pair-1 filler: its remaining projection chains in 4-matmul units,
        # spread across the chunks of the preceding q-block (each unit must
        # land before its stage starts: stages begin at chunks 4, 12, 24)
        fill1 = {}
        for ci, (tb, ft_kind, half) in [
                (0, (1, 0, 0)), (1, (1, 0, 1)), (2, (1, 1, 0)), (3, (1, 1, 1)),
                (5, (2, 0, 0)), (7, (2, 0, 1)), (9, (2, 1, 0)), (11, (2, 1, 1)),
                (13, (3, 0, 0)), (16, (3, 0, 1)), (19, (3, 1, 0)),
                (22, (3, 1, 1))]:
            fill1.setdefault(ci, []).append(funit(tb, ft_kind, half))

        run_pair(0, stage0, fill0, flip_from=3)
        run_pair(1, {}, fill1, flip_from=0)

    nc.compile()
    return nc


def _get_nc():
    global _NC
    if _NC is None:
        _NC = _build()
    return _NC


def _make_in_maps(x, W_attn):
    x = np.asarray(x, dtype=np.float32)
    W = np.asarray(W_attn, dtype=np.float32)
    wq, wk, wv = W[0:C], W[C:2 * C], W[2 * C:3 * C]
    bf = ml_dtypes.bfloat16
    in_maps = []
    for c in range(NCORES):
        b, g = c // 4, c % 4
        heads = [HPC * g + i for i in range(HPC)]
        xTb = np.ascontiguousarray(x[b].T).astype(bf)
        qrows = np.concatenate([wq[D * h:D * h + D] for h in heads], axis=0)
        krows = np.concatenate([wk[D * h:D * h + D] for h in heads], axis=0)
        wqk_np = np.concatenate([qrows, krows], 0).T  # [C, 512]
        wv_np = np.zeros((C, HPC * 65), np.float32)
        for i, h in enumerate(heads):
            wv_np[:, 65 * i:65 * i + D] = wv[D * h:D * h + D].T
        # pre-tile on the partition axis so each weight loads in O(1)
        # dma_starts: wqk becomes [p, ft*1024 + c*128 + f] (ft-major, so
        # the first chain's q weights are a small leading transfer), wv
        # becomes [p, c*260 + f]
        wqk_t = np.ascontiguousarray(
            wqk_np.reshape(8, 128, 4, 128).transpose(1, 2, 0, 3)
            .reshape(128, -1))
        wv_t = np.ascontiguousarray(
            wv_np.reshape(8, 128, 260).transpose(1, 0, 2).reshape(128, -1))
        in_maps.append({"xT": xTb, "wqk": wqk_t.astype(bf),
                        "wv": wv_t.astype(bf)})
    return in_maps


def _execute(in_maps, trace=False):
    return run_bass_kernel_spmd(_get_nc(), in_maps,
                                core_ids=list(range(NCORES)), trace=trace)


def _assemble(results):
    y = np.empty((B, T, C), np.float32)
    for c in range(NCORES):
        b, g = c // 4, c % 4
        # [HPC, 65, T] bf16; row 64 = softmax denominator
        yc = results[c]["y"].astype(np.float32)
        for i in range(HPC):
            h = HPC * g + i
            y[b, :, D * h:D * h + D] = (yc[i, 0:64] / yc[i, 64:65]).T
    return y


def kernel(x, W_attn):
    res = _execute(_make_in_maps(x, W_attn), trace=False)
    return _assemble(res.results)


# revision 34
# speedup vs baseline: 1.0065x; 1.0065x over previous
"""Causal self-attention (B=2, T=2048, C=1024, H=16) on 8 TRN2 NeuronCores.

Sharding: core c handles batch b = c//4 and heads 4*(c%4) .. 4*(c%4)+3
(data-parallel over B, tensor-parallel over heads; full K/V for its heads
is computed locally from the core's QKV projection slice).

Optimizations over the f32r baseline (185us -> ~126us):
  - all matmul operands bf16 (separate LDWEIGHTS with FWL + pull-ahead
    instead of f32r's self-loading weight path; no f32r N<256 4x penalty;
    half the DMA bytes); PSUM accumulation stays f32
  - the two K=64 S-matmuls of a head pair land in disjoint PE row groups
    (auto tile_position from base partitions 0/64) and execute
    concurrently (second MM ends ~3ns after the first)
  - exp only over causally-valid columns; causal mask shrunk to the one
    128-column staircase band per diagonal tile (PV skips dead columns,
    so only the band needs zeroing) -> ~4x less gpsimd and a shorter
    exp->mask->PV critical path
  - softmax division done on the host during unshard (row 64 of each
    head's output carries the denominators); kills the on-device
    reciprocal/broadcast/multiply epilogue and its serial tail
  - startup was DMA descriptor-gen bound (~0.65us per dma_start,
    serialized per HWDGE ring): weights arrive pre-tiled so all input
    lands in 9 dma_starts issued in strict need-order, and the HAM
    warm-up bridges until the first chain's data arrives
  - projection chains are spliced between attention chunks in 4-matmul
    units so the in-order PE always has ready work while ScalarE grinds
    exp (the attention steady state is exp-bound at ~1.0us/chunk)

Per-core dataflow:
  - host passes xT = x[b].T [C,T] bf16, wqk [128, ft*1024+c*128+f] bf16
    (ft-major pre-tiled q/k weights), wv [128, c*260+f] bf16 (per head a
    65-wide block whose last column is zero)
  - qT/kT [64,T] per head via projection matmuls (contraction c on
    partitions), PSUM f32, cast to bf16 on the PSUM->SBUF copy
  - v [t,260] bf16 with a ones column appended per head (65th of a block)
  - head pairs (2p, 2p+1) share S^T tiles: s [k=128, 1024] = [S_even|S_odd]
    f32 PSUM, exp on ScalarE (scale=1/8 fused) -> bf16 pt
  - y^T [65, 512] += V'.T @ P^T accumulated over k-tiles; row 64 = softmax
    denominators (from the ones column)
  - y^T copied PSUM->SBUF bf16 (vector/scalar alternating), DMA'd out;
    host divides rows 0:64 by row 64, transposes, concats heads.
"""

import os
import sys
import types
import numpy as np
import ml_dtypes

import concourse.bass as bass
import concourse.mybir as mybir
import concourse.tile as tile
from concourse import bacc
from concourse.bass_utils import run_bass_kernel_spmd

B, T, C, H = 2, 2048, 1024, 16
D = 64
NCORES = 8
HPC = 4          # heads per core
NQB = 4          # q blocks of 512
QB = 512
F32 = mybir.dt.float32
BF16 = mybir.dt.bfloat16
EXP = mybir.ActivationFunctionType.Exp
IS_GE = mybir.AluOpType.is_ge


def _install_profhook():
    """Register the NTFF profile hook shim so BASS_TRACE=1 works; harmless
    no-op (graceful trace skip) when the axon .so lacks profiling."""
    if "antenv.axon_hooks" not in sys.modules:
        mod = types.ModuleType("antenv.axon_hooks")
        mod._hook = None
        mod.set_axon_ntff_profile_hook = lambda h: setattr(mod, "_hook", h)
        mod.get_axon_ntff_profile_hook = lambda: mod._hook
        sys.modules["antenv.axon_hooks"] = mod
        try:
            import antenv
            antenv.axon_hooks = mod
        except ImportError:
            pass
    try:
        from trn_agent_boot.trn_boot import _ntff_profile_via_ctypes
        sys.modules["antenv.axon_hooks"].set_axon_ntff_profile_hook(
            _ntff_profile_via_ctypes("/opt/axon/libaxon_pjrt.so")
        )
        import concourse.bass_utils as bu
        bu.upload_artifacts = lambda tmpdir: tmpdir
    except Exception:
        pass


_install_profhook()

_NC = None


def _build():
    nc = bacc.Bacc("TRN2", target_bir_lowering=False, debug=False,
                   num_devices=NCORES)
    # weights arrive pre-tiled on the partition axis (row p holds c-tile
    # slice [c*128+p, :] at columns c*W..) so each loads as ONE dma_start:
    # descriptor generation is ~0.65us per dma_start regardless of size,
    # and the startup was descriptor-gen bound
    xT_d = nc.declare_dram_parameter("xT", [C, T], BF16, isOutput=False)
    wqk_d = nc.declare_dram_parameter("wqk", [128, 4 * 8 * 128], BF16,
                                      isOutput=False)
    wv_d = nc.declare_dram_parameter("wv", [128, 8 * 260], BF16,
                                     isOutput=False)
    y_d = nc.declare_dram_parameter("y", [HPC, 65, T], BF16, isOutput=True)

    from contextlib import ExitStack
    with tile.TileContext(nc) as tc, ExitStack() as ctx:
        sb = ctx.enter_context(tc.tile_pool(name="sb", bufs=1))
        pp = ctx.enter_context(tc.tile_pool(name="pp", bufs=8))
        yp = ctx.enter_context(tc.tile_pool(name="yp", bufs=3))
        psp = ctx.enter_context(tc.tile_pool(name="psp", bufs=2, space="PSUM"))
        pss = ctx.enter_context(tc.tile_pool(name="pss", bufs=2, space="PSUM"))
        psy = ctx.enter_context(tc.tile_pool(name="psy", bufs=1, space="PSUM"))

        # x^T merged per t-block: xall[tb][p, c*512+f] = x[b].T[c*128+p,
        # tb*512+f] — each t-block loads as ONE dma_start (startup is both
        # descriptor-gen and bandwidth bound, so fewest gens in strict
        # need-order wins). wqk is ft-major so the first q-chain's weights
        # are their own small transfer.
        xall = [sb.tile([128, 8 * 512], BF16, name=f"xall{tb}")
                for tb in range(4)]
        wqks_all = sb.tile([128, 4 * 8 * 128], BF16, name="wqks")
        wvs_all = sb.tile([128, 8 * 260], BF16, name="wvs")
        qs = [[sb.tile([128, 512], BF16, name=f"q{p}_{tb}") for tb in range(4)]
              for p in range(2)]
        ks = [[sb.tile([128, 512], BF16, name=f"k{p}_{tb}") for tb in range(4)]
              for p in range(2)]
        vs = [sb.tile([128, 260], BF16, name=f"v_{t}") for t in range(16)]
        ones2 = sb.tile([128, 4], F32, name="ones2")
        nc.gpsimd.memset(ones2[:], 1.0)

        def xslice(tb, lo=0, hi=512):
            """AP slice of x^T covering t-block tb, contraction tile c."""
            return lambda c: xall[tb][:, c * 512 + lo:c * 512 + hi]

        # warm-up: keep the PE's HAM activity monitor busy while the input
        # DMAs land, so real matmuls start at 2.4 GHz instead of 1.2 GHz.
        # The operand memset rides the vector engine (gpsimd wakes ~3 us
        # later), and 22 cold matmuls (~4.7 us) bridge until the first
        # projection chain's inputs have arrived.
        wup = sb.tile([128, 256], BF16, name="wup")
        nc.vector.memset(wup[:], 0.5)
        wups = psp.tile([128, 256], F32, name="wups", tag="pmm")
        for _ in range(22):
            nc.tensor.matmul(wups[:], wup[:, 0:128], wup[:], start=True,
                             stop=True)

        # 8 dma_starts in strict need-order: q-pair0 weights (0.25MB), x
        # t-block 0 (1MB), k-pair0 weights, wv, pair-1 qk weights (one
        # strided transfer), then x t-blocks 1..3
        xT3 = xT_d.ap().rearrange("(c p) t -> p c t", c=8)  # [128, 8, 2048]

        def dma_xall(tb, c0=0, c1=8):
            nc.sync.dma_start(
                xall[tb][:, c0 * 512:c1 * 512].rearrange(
                    "p (c f) -> p c f", c=c1 - c0),
                xT3[:, c0:c1, tb * 512:(tb + 1) * 512])

        nc.sync.dma_start(wqks_all[:, 0:1024], wqk_d.ap()[:, 0:1024])
        # t-block 0 in halves so the first chain's c=0..3 matmuls can
        # start while c=4..7 is still in flight
        dma_xall(0, 0, 4)
        dma_xall(0, 4, 8)
        nc.sync.dma_start(wqks_all[:, 2048:3072], wqk_d.ap()[:, 2048:3072])
        nc.sync.dma_start(wvs_all[:], wv_d.ap()[:, :])
        nc.sync.dma_start(
            wqks_all[:].rearrange("p (b x) -> p b x", b=2)[:, :, 1024:2048],
            wqk_d.ap().rearrange("p (b x) -> p b x", b=2)[:, :, 1024:2048])
        for tb in (1, 2, 3):
            dma_xall(tb)

        qk_cache = {}

        def qk_chain_part(p, ft_kind, tb, half):
            """Half of one projection chain (4 accumulating matmuls); the
            second half finishes the accumulation and casts PSUM->SBUF."""
            ft = p if ft_kind == 0 else 2 + p
            key = (p, ft_kind, tb)
            if half == 0:
                qk_cache[key] = psp.tile([128, 512], F32,
                                         name=f"pqk{p}_{ft}_{tb}", tag="pmm")
            mm = qk_cache[key]
            xs = xslice(tb)
            for c in range(4 * half, 4 * half + 4):
                nc.tensor.matmul(mm[:],
                                 wqks_all[:, ft * 1024 + c * 128:
                                          ft * 1024 + (c + 1) * 128],
                                 xs(c),
                                 start=(c == 0), stop=(c == 7))
            if half == 1:
                dst = (qs if ft_kind == 0 else ks)[p][tb]
                nc.vector.tensor_copy(dst[:], mm[:])
                del qk_cache[key]

        def qk_chain(p, ft_kind, tb):
            qk_chain_part(p, ft_kind, tb, 0)
            qk_chain_part(p, ft_kind, tb, 1)

        def v_chain(tt):
            """Combined v projection for one t-tile (all 4 heads, N=260)."""
            tb, sub = tt // 4, tt % 4
            mmv = psp.tile([128, 260], F32, name=f"pv{tt}", tag="pmm")
            xs = xslice(tb, sub * 128, (sub + 1) * 128)
            for c in range(8):
                nc.tensor.matmul(mmv[:], xs(c),
                                 wvs_all[:, c * 260:(c + 1) * 260],
                                 start=(c == 0), stop=(c == 7))
            nc.vector.tensor_copy(vs[tt][:], mmv[:])
            nc.vector.tensor_copy(vs[tt][:, 64:260:65], ones2[:])

        def attn_s_part(p, j, kk, ptiles):
            """S matmuls + exp + causal mask for chunk (p, j, kk).

            Diagonal k-tiles only have valid scores for q >= k, i.e. local
            q >= off = 128*(kk-4j); matmuls and exp skip the dead columns
            (PV skips them too, so they can hold stale garbage), and only
            the 128-wide staircase band [off, off+128) needs masking."""
            off = max(0, 128 * (kk - 4 * j))
            s = pss.tile([128, 1024], F32, name=f"s{p}_{j}_{kk}", tag="s")
            ktb, ksub = kk // 4, (kk % 4) * 128
            nc.tensor.matmul(s[:, off:512],
                             ks[p][ktb][0:64, ksub:ksub + 128],
                             qs[p][j][0:64, off:512],
                             start=True, stop=True)
            nc.tensor.matmul(s[:, 512 + off:1024],
                             ks[p][ktb][64:128, ksub:ksub + 128],
                             qs[p][j][64:128, off:512],
                             start=True, stop=True)
            pt = pp.tile([128, 1024], BF16, name=f"pt{p}_{j}_{kk}", tag="pt")
            if off:
                nc.scalar.activation(
                    pt[:].rearrange("p (b q) -> p b q", b=2)[:, :, off:512],
                    s[:].rearrange("p (b q) -> p b q", b=2)[:, :, off:512],
                    EXP, scale=0.125)
            else:
                nc.scalar.activation(pt[:], s[:], EXP, scale=0.125)
            if kk >= 4 * j:
                # zero P where q < k; only the staircase band straddles the
                # diagonal (cols [off, off+128) of both head halves); the
                # predicate reduces to local_q >= key_partition (base=0)
                band = pt[:].rearrange("p (b q) -> p b q", b=2)[
                    :, :, off:off + 128]
                nc.gpsimd.affine_select(
                    band, band,
                    pattern=[[0, 2], [1, 128]],
                    compare_op=IS_GE, fill=0.0,
                    base=0,
                    channel_multiplier=-1)
            ptiles[(j, kk)] = pt

        def emit_out(p, j, state):
            """Copy the finished y^T accumulators (incl. denominator row 64)
            PSUM->SBUF as bf16 and DMA out in 4 row-chunks (spread across
            DMA queues); host does the divide. The very last output's
            second copy rides ScalarE (idle by then) so the two tail
            copies run in parallel; midstream both stay off ScalarE,
            which is the attention bottleneck (gpsimd cannot read PSUM)."""
            for h01, key in ((0, "ye"), (1, "yo")):
                ysb = yp.tile([65, 512], BF16,
                              name=f"ysb{p}_{j}_{h01}", tag="ysb")
                last = h01 == 1 and p == 1 and j == NQB - 1
                if last:
                    nc.scalar.copy(ysb[:], state[key][:])
                else:
                    nc.vector.tensor_copy(ysb[:], state[key][:])
                # one dma_start per head tile (the HW splits it across all
                # 16 SDMA engines); the final tile rides the scalar HWDGE
                # ring so the two tail DMAs drain in parallel
                eng = nc.scalar if last else nc.sync
                eng.dma_start(
                    y_d.ap()[2 * p + h01, :, j * 512:(j + 1) * 512], ysb[:])

        def attn_pv_part(p, j, kk, state, ptiles):
            """PV-accumulation closures for chunk (p, j, kk): one matmul per
            head half (so the caller can group same-PSUM-bank matmuls), plus
            a trailing output closure on the q-block's last k-tile."""
            nkt = 4 * (j + 1)
            if kk == 0:
                state["ye"] = psy.tile([65, 512], F32,
                                       name=f"ye{p}_{j}", tag="ye")
                state["yo"] = psy.tile([65, 512], F32,
                                       name=f"yo{p}_{j}", tag="yo")
            pt = ptiles.pop((j, kk))
            first, last = (kk == 0), (kk == nkt - 1)
            # skip columns where P is all-zero (above the causal diagonal);
            # their y contribution is zero and PSUM keeps the prior partials
            off = 0 if first else max(0, 128 * (kk - 4 * j))

            def mm_e():
                nc.tensor.matmul(state["ye"][:, off:512],
                                 vs[kk][:, 130 * p:130 * p + 65],
                                 pt[:, off:512],
                                 start=first, stop=last)

            def mm_o():
                nc.tensor.matmul(state["yo"][:, off:512],
                                 vs[kk][:, 130 * p + 65:130 * p + 130],
                                 pt[:, 512 + off:1024],
                                 start=first, stop=last)

            fin = (lambda: emit_out(p, j, state)) if last else None
            return mm_e, mm_o, fin

        ptiles = {}
        states = {}

        def run_pair(p, stage_work, filler=None):
            """Emit the pair's attention as one flat pipeline in batches of
            two chunks: S/exp of batch b is emitted before PV of batch b-1
            (across q-block boundaries), so the in-order PE never stalls
            behind exp. stage_work (projection chains) is emitted at a
            q-block's first chunk; filler[i] work units are spliced in
            right after chunk i's S matmuls (PE food during exp waits)."""
            seq = [(j, kk) for j in range(NQB) for kk in range(4 * (j + 1))]
            batches = [seq[i:i + 2] for i in range(0, len(seq), 2)]
            filler = filler or {}

            def emit_pv(batch):
                parts = [attn_pv_part(p, pj, pkk,
                                      states.setdefault((p, pj), {}), ptiles)
                         for (pj, pkk) in batch]
                for e, o, _ in parts:
                    e()
                    o()
                for _, _, fin in parts:
                    if fin is not None:
                        fin()

            prev = None
            ci = 0
            for batch in batches:
                for (j, kk) in batch:
                    if kk == 0:
                        for w in stage_work.get(j, ()):
                            w()
                    attn_s_part(p, j, kk, ptiles)
                    for w in filler.get(ci, ()):
                        w()
                    ci += 1
                if prev is not None:
                    emit_pv(prev)
                prev = batch
            emit_pv(prev)

        # pair-0 stages: its own q/k projections + the first v tile of the
        # block; the remaining v tiles and pair 1's t-block-0 projections
        # are spliced between chunks as filler (q-blocks start at chunk
        # indices 0, 4, 12, 24; v(4j+i) must land ~i chunks in, before the
        # PV that consumes it)
        stage0 = {}
        for j in range(NQB):
            stage0[j] = [lambda j=j: qk_chain(0, 0, j),
                         lambda j=j: qk_chain(0, 1, j)]
            if j:
                stage0[j].append(lambda j=j: v_chain(4 * j))
        fill0 = {0: [lambda: v_chain(0)]}
        for j, base in enumerate((0, 4, 12, 24)):
            for i in (1, 2, 3):
                fill0.setdefault(base + i - 1, []).append(
                    lambda tt=4 * j + i: v_chain(tt))

        def funit(tb, ft_kind, half):
            return lambda: qk_chain_part(1, ft_kind, tb, half)

        for ci, (tb, ft_kind, half) in [
                (27, (0, 0, 0)), (29, (0, 0, 1)),
                (31, (0, 1, 0)), (33, (0, 1, 1))]:
            fill0.setdefault(ci, []).append(funit(tb, ft_kind, half))

        # pair-1 filler: its remaining projection chains in 4-matmul units,
        # spread across the chunks of the preceding q-block (each unit must
        # land before its stage starts: stages begin at chunks 4, 12, 24)
        fill1 = {}
        for ci, (tb, ft_kind, half) in [
                (0, (1, 0, 0)), (1, (1, 0, 1)), (2, (1, 1, 0)), (3, (1, 1, 1)),
                (5, (2, 0, 0)), (7, (2, 0, 1)), (9, (2, 1, 0)), (11, (2, 1, 1)),
                (13, (3, 0, 0)), (16, (3, 0, 1)), (19, (3, 1, 0)),
                (22, (3, 1, 1))]:
            fill1.setdefault(ci, []).append(funit(tb, ft_kind, half))

        run_pair(0, stage0, fill0)
        run_pair(1, {}, fill1)

    nc.compile()
    return nc


def _get_nc():
    global _NC
    if _NC is None:
        _NC = _build()
    return _NC


def _make_in_maps(x, W_attn):
    x = np.asarray(x, dtype=np.float32)
    W = np.asarray(W_attn, dtype=np.float32)
    wq, wk, wv = W[0:C], W[C:2 * C], W[2 * C:3 * C]
    bf = ml_dtypes.bfloat16
    in_maps = []
    for c in range(NCORES):
        b, g = c // 4, c % 4
        heads = [HPC * g + i for i in range(HPC)]
        xTb = np.ascontiguousarray(x[b].T).astype(bf)
        qrows = np.concatenate([wq[D * h:D * h + D] for h in heads], axis=0)
        krows = np.concatenate([wk[D * h:D * h + D] for h in heads], axis=0)
        wqk_np = np.concatenate([qrows, krows], 0).T  # [C, 512]
        wv_np = np.zeros((C, HPC * 65), np.float32)
        for i, h in enumerate(heads):
            wv_np[:, 65 * i:65 * i + D] = wv[D * h:D * h + D].T
        # pre-tile on the partition axis so each weight loads in O(1)
        # dma_starts: wqk becomes [p, ft*1024 + c*128 + f] (ft-major, so
        # the first chain's q weights are a small leading transfer), wv
        # becomes [p, c*260 + f]
        wqk_t = np.ascontiguousarray(
            wqk_np.reshape(8, 128, 4, 128).transpose(1, 2, 0, 3)
            .reshape(128, -1))
        wv_t = np.ascontiguousarray(
            wv_np.reshape(8, 128, 260).transpose(1, 0, 2).reshape(128, -1))
        in_maps.append({"xT": xTb, "wqk": wqk_t.astype(bf),
                        "wv": wv_t.astype(bf)})
    return in_maps


def _execute(in_maps, trace=False):
    return run_bass_kernel_spmd(_get_nc(), in_maps,
                                core_ids=list(range(NCORES)), trace=trace)


def _assemble(results):
    y = np.empty((B, T, C), np.float32)
    for c in range(NCORES):
        b, g = c // 4, c % 4
        # [HPC, 65, T] bf16; row 64 = softmax denominator
        yc = results[c]["y"].astype(np.float32)
        for i in range(HPC):
            h = HPC * g + i
            y[b, :, D * h:D * h + D] = (yc[i, 0:64] / yc[i, 64:65]).T
    return y


def kernel(x, W_attn):
    res = _execute(_make_in_maps(x, W_attn), trace=False)
    return _assemble(res.results)


# revision 35
# speedup vs baseline: 1.0109x; 1.0044x over previous
"""Causal self-attention (B=2, T=2048, C=1024, H=16) on 8 TRN2 NeuronCores.

Sharding: core c handles batch b = c//4 and heads 4*(c%4) .. 4*(c%4)+3
(data-parallel over B, tensor-parallel over heads; full K/V for its heads
is computed locally from the core's QKV projection slice).

Optimizations over the f32r baseline (185us -> ~126us):
  - all matmul operands bf16 (separate LDWEIGHTS with FWL + pull-ahead
    instead of f32r's self-loading weight path; no f32r N<256 4x penalty;
    half the DMA bytes); PSUM accumulation stays f32
  - the two K=64 S-matmuls of a head pair land in disjoint PE row groups
    (auto tile_position from base partitions 0/64) and execute
    concurrently (second MM ends ~3ns after the first)
  - exp only over causally-valid columns; causal mask shrunk to the one
    128-column staircase band per diagonal tile (PV skips dead columns,
    so only the band needs zeroing) -> ~4x less gpsimd and a shorter
    exp->mask->PV critical path
  - softmax division done on the host during unshard (row 64 of each
    head's output carries the denominators); kills the on-device
    reciprocal/broadcast/multiply epilogue and its serial tail
  - startup was DMA descriptor-gen bound (~0.65us per dma_start,
    serialized per HWDGE ring): weights arrive pre-tiled so all input
    lands in 9 dma_starts issued in strict need-order, and the HAM
    warm-up bridges until the first chain's data arrives
  - projection chains are spliced between attention chunks in 4-matmul
    units so the in-order PE always has ready work while ScalarE grinds
    exp (the attention steady state is exp-bound at ~1.0us/chunk)

Per-core dataflow:
  - host passes xT = x[b].T [C,T] bf16, wqk [128, ft*1024+c*128+f] bf16
    (ft-major pre-tiled q/k weights), wv [128, c*260+f] bf16 (per head a
    65-wide block whose last column is zero)
  - qT/kT [64,T] per head via projection matmuls (contraction c on
    partitions), PSUM f32, cast to bf16 on the PSUM->SBUF copy
  - v [t,260] bf16 with a ones column appended per head (65th of a block)
  - head pairs (2p, 2p+1) share S^T tiles: s [k=128, 1024] = [S_even|S_odd]
    f32 PSUM, exp on ScalarE (scale=1/8 fused) -> bf16 pt
  - y^T [65, 512] += V'.T @ P^T accumulated over k-tiles; row 64 = softmax
    denominators (from the ones column)
  - y^T copied PSUM->SBUF bf16 (vector/scalar alternating), DMA'd out;
    host divides rows 0:64 by row 64, transposes, concats heads.
"""

import os
import sys
import types
import numpy as np
import ml_dtypes

import concourse.bass as bass
import concourse.mybir as mybir
import concourse.tile as tile
from concourse import bacc
from concourse.bass_utils import run_bass_kernel_spmd

B, T, C, H = 2, 2048, 1024, 16
D = 64
NCORES = 8
HPC = 4          # heads per core
NQB = 4          # q blocks of 512
QB = 512
F32 = mybir.dt.float32
BF16 = mybir.dt.bfloat16
EXP = mybir.ActivationFunctionType.Exp
IS_GE = mybir.AluOpType.is_ge


def _install_profhook():
    """Register the NTFF profile hook shim so BASS_TRACE=1 works; harmless
    no-op (graceful trace skip) when the axon .so lacks profiling."""
    if "antenv.axon_hooks" not in sys.modules:
        mod = types.ModuleType("antenv.axon_hooks")
        mod._hook = None
        mod.set_axon_ntff_profile_hook = lambda h: setattr(mod, "_hook", h)
        mod.get_axon_ntff_profile_hook = lambda: mod._hook
        sys.modules["antenv.axon_hooks"] = mod
        try:
            import antenv
            antenv.axon_hooks = mod
        except ImportError:
            pass
    try:
        from trn_agent_boot.trn_boot import _ntff_profile_via_ctypes
        sys.modules["antenv.axon_hooks"].set_axon_ntff_profile_hook(
            _ntff_profile_via_ctypes("/opt/axon/libaxon_pjrt.so")
        )
        import concourse.bass_utils as bu
        bu.upload_artifacts = lambda tmpdir: tmpdir
    except Exception:
        pass


_install_profhook()

_NC = None


def _build():
    nc = bacc.Bacc("TRN2", target_bir_lowering=False, debug=False,
                   num_devices=NCORES)
    # weights arrive pre-tiled on the partition axis (row p holds c-tile
    # slice [c*128+p, :] at columns c*W..) so each loads as ONE dma_start:
    # descriptor generation is ~0.65us per dma_start regardless of size,
    # and the startup was descriptor-gen bound
    xT_d = nc.declare_dram_parameter("xT", [C, T], BF16, isOutput=False)
    wqk_d = nc.declare_dram_parameter("wqk", [128, 4 * 8 * 128], BF16,
                                      isOutput=False)
    wv_d = nc.declare_dram_parameter("wv", [128, 8 * 260], BF16,
                                     isOutput=False)
    y_d = nc.declare_dram_parameter("y", [HPC, 65, T], BF16, isOutput=True)

    from contextlib import ExitStack
    with tile.TileContext(nc) as tc, ExitStack() as ctx:
        sb = ctx.enter_context(tc.tile_pool(name="sb", bufs=1))
        pp = ctx.enter_context(tc.tile_pool(name="pp", bufs=8))
        yp = ctx.enter_context(tc.tile_pool(name="yp", bufs=3))
        psp = ctx.enter_context(tc.tile_pool(name="psp", bufs=2, space="PSUM"))
        pss = ctx.enter_context(tc.tile_pool(name="pss", bufs=2, space="PSUM"))
        psy = ctx.enter_context(tc.tile_pool(name="psy", bufs=1, space="PSUM"))

        # x^T merged per t-block: xall[tb][p, c*512+f] = x[b].T[c*128+p,
        # tb*512+f] — each t-block loads as ONE dma_start (startup is both
        # descriptor-gen and bandwidth bound, so fewest gens in strict
        # need-order wins). wqk is ft-major so the first q-chain's weights
        # are their own small transfer.
        xall = [sb.tile([128, 8 * 512], BF16, name=f"xall{tb}")
                for tb in range(4)]
        wqks_all = sb.tile([128, 4 * 8 * 128], BF16, name="wqks")
        wvs_all = sb.tile([128, 8 * 260], BF16, name="wvs")
        qs = [[sb.tile([128, 512], BF16, name=f"q{p}_{tb}") for tb in range(4)]
              for p in range(2)]
        ks = [[sb.tile([128, 512], BF16, name=f"k{p}_{tb}") for tb in range(4)]
              for p in range(2)]
        vs = [sb.tile([128, 260], BF16, name=f"v_{t}") for t in range(16)]
        ones2 = sb.tile([128, 4], F32, name="ones2")
        nc.gpsimd.memset(ones2[:], 1.0)

        def xslice(tb, lo=0, hi=512):
            """AP slice of x^T covering t-block tb, contraction tile c."""
            return lambda c: xall[tb][:, c * 512 + lo:c * 512 + hi]

        # warm-up: keep the PE's HAM activity monitor busy while the input
        # DMAs land, so real matmuls start at 2.4 GHz instead of 1.2 GHz.
        # The operand memset rides the vector engine (gpsimd wakes ~3 us
        # later), and 40 matmuls (~17 cold then warm, ~6.3 us) bridge until the first
        # projection chain's inputs have arrived.
        wup = sb.tile([128, 256], BF16, name="wup")
        nc.vector.memset(wup[:], 0.5)
        wups = psp.tile([128, 256], F32, name="wups", tag="pmm")
        for _ in range(40):
            nc.tensor.matmul(wups[:], wup[:, 0:128], wup[:], start=True,
                             stop=True)

        # 8 dma_starts in strict need-order: q-pair0 weights (0.25MB), x
        # t-block 0 (1MB), k-pair0 weights, wv, pair-1 qk weights (one
        # strided transfer), then x t-blocks 1..3
        xT3 = xT_d.ap().rearrange("(c p) t -> p c t", c=8)  # [128, 8, 2048]

        def dma_xall(tb, c0=0, c1=8):
            nc.sync.dma_start(
                xall[tb][:, c0 * 512:c1 * 512].rearrange(
                    "p (c f) -> p c f", c=c1 - c0),
                xT3[:, c0:c1, tb * 512:(tb + 1) * 512])

        nc.sync.dma_start(wqks_all[:, 0:1024], wqk_d.ap()[:, 0:1024])
        # t-block 0 in halves so the first chain's c=0..3 matmuls can
        # start while c=4..7 is still in flight
        dma_xall(0, 0, 4)
        dma_xall(0, 4, 8)
        nc.sync.dma_start(wqks_all[:, 2048:3072], wqk_d.ap()[:, 2048:3072])
        nc.sync.dma_start(wvs_all[:], wv_d.ap()[:, :])
        nc.sync.dma_start(
            wqks_all[:].rearrange("p (b x) -> p b x", b=2)[:, :, 1024:2048],
            wqk_d.ap().rearrange("p (b x) -> p b x", b=2)[:, :, 1024:2048])
        for tb in (1, 2, 3):
            dma_xall(tb)

        qk_cache = {}

        def qk_chain_part(p, ft_kind, tb, half):
            """Half of one projection chain (4 accumulating matmuls); the
            second half finishes the accumulation and casts PSUM->SBUF."""
            ft = p if ft_kind == 0 else 2 + p
            key = (p, ft_kind, tb)
            if half == 0:
                qk_cache[key] = psp.tile([128, 512], F32,
                                         name=f"pqk{p}_{ft}_{tb}", tag="pmm")
            mm = qk_cache[key]
            xs = xslice(tb)
            for c in range(4 * half, 4 * half + 4):
                nc.tensor.matmul(mm[:],
                                 wqks_all[:, ft * 1024 + c * 128:
                                          ft * 1024 + (c + 1) * 128],
                                 xs(c),
                                 start=(c == 0), stop=(c == 7))
            if half == 1:
                dst = (qs if ft_kind == 0 else ks)[p][tb]
                nc.vector.tensor_copy(dst[:], mm[:])
                del qk_cache[key]

        def qk_chain(p, ft_kind, tb):
            qk_chain_part(p, ft_kind, tb, 0)
            qk_chain_part(p, ft_kind, tb, 1)

        def v_chain(tt):
            """Combined v projection for one t-tile (all 4 heads, N=260)."""
            tb, sub = tt // 4, tt % 4
            mmv = psp.tile([128, 260], F32, name=f"pv{tt}", tag="pmm")
            xs = xslice(tb, sub * 128, (sub + 1) * 128)
            for c in range(8):
                nc.tensor.matmul(mmv[:], xs(c),
                                 wvs_all[:, c * 260:(c + 1) * 260],
                                 start=(c == 0), stop=(c == 7))
            nc.vector.tensor_copy(vs[tt][:], mmv[:])
            nc.vector.tensor_copy(vs[tt][:, 64:260:65], ones2[:])

        def attn_s_part(p, j, kk, ptiles):
            """S matmuls + exp + causal mask for chunk (p, j, kk).

            Diagonal k-tiles only have valid scores for q >= k, i.e. local
            q >= off = 128*(kk-4j); matmuls and exp skip the dead columns
            (PV skips them too, so they can hold stale garbage), and only
            the 128-wide staircase band [off, off+128) needs masking."""
            off = max(0, 128 * (kk - 4 * j))
            s = pss.tile([128, 1024], F32, name=f"s{p}_{j}_{kk}", tag="s")
            ktb, ksub = kk // 4, (kk % 4) * 128
            nc.tensor.matmul(s[:, off:512],
                             ks[p][ktb][0:64, ksub:ksub + 128],
                             qs[p][j][0:64, off:512],
                             start=True, stop=True)
            nc.tensor.matmul(s[:, 512 + off:1024],
                             ks[p][ktb][64:128, ksub:ksub + 128],
                             qs[p][j][64:128, off:512],
                             start=True, stop=True)
            pt = pp.tile([128, 1024], BF16, name=f"pt{p}_{j}_{kk}", tag="pt")
            if off:
                nc.scalar.activation(
                    pt[:].rearrange("p (b q) -> p b q", b=2)[:, :, off:512],
                    s[:].rearrange("p (b q) -> p b q", b=2)[:, :, off:512],
                    EXP, scale=0.125)
            else:
                nc.scalar.activation(pt[:], s[:], EXP, scale=0.125)
            if kk >= 4 * j:
                # zero P where q < k; only the staircase band straddles the
                # diagonal (cols [off, off+128) of both head halves); the
                # predicate reduces to local_q >= key_partition (base=0)
                band = pt[:].rearrange("p (b q) -> p b q", b=2)[
                    :, :, off:off + 128]
                nc.gpsimd.affine_select(
                    band, band,
                    pattern=[[0, 2], [1, 128]],
                    compare_op=IS_GE, fill=0.0,
                    base=0,
                    channel_multiplier=-1)
            ptiles[(j, kk)] = pt

        def emit_out(p, j, state):
            """Copy the finished y^T accumulators (incl. denominator row 64)
            PSUM->SBUF as bf16 and DMA out in 4 row-chunks (spread across
            DMA queues); host does the divide. The very last output's
            second copy rides ScalarE (idle by then) so the two tail
            copies run in parallel; midstream both stay off ScalarE,
            which is the attention bottleneck (gpsimd cannot read PSUM)."""
            for h01, key in ((0, "ye"), (1, "yo")):
                ysb = yp.tile([65, 512], BF16,
                              name=f"ysb{p}_{j}_{h01}", tag="ysb")
                last = h01 == 1 and p == 1 and j == NQB - 1
                if last:
                    nc.scalar.copy(ysb[:], state[key][:])
                else:
                    nc.vector.tensor_copy(ysb[:], state[key][:])
                # one dma_start per head tile (the HW splits it across all
                # 16 SDMA engines); the final tile rides the scalar HWDGE
                # ring so the two tail DMAs drain in parallel
                eng = nc.scalar if last else nc.sync
                eng.dma_start(
                    y_d.ap()[2 * p + h01, :, j * 512:(j + 1) * 512], ysb[:])

        def attn_pv_part(p, j, kk, state, ptiles):
            """PV-accumulation closures for chunk (p, j, kk): one matmul per
            head half (so the caller can group same-PSUM-bank matmuls), plus
            a trailing output closure on the q-block's last k-tile."""
            nkt = 4 * (j + 1)
            if kk == 0:
                state["ye"] = psy.tile([65, 512], F32,
                                       name=f"ye{p}_{j}", tag="ye")
                state["yo"] = psy.tile([65, 512], F32,
                                       name=f"yo{p}_{j}", tag="yo")
            pt = ptiles.pop((j, kk))
            first, last = (kk == 0), (kk == nkt - 1)
            # skip columns where P is all-zero (above the causal diagonal);
            # their y contribution is zero and PSUM keeps the prior partials
            off = 0 if first else max(0, 128 * (kk - 4 * j))

            def mm_e():
                nc.tensor.matmul(state["ye"][:, off:512],
                                 vs[kk][:, 130 * p:130 * p + 65],
                                 pt[:, off:512],
                                 start=first, stop=last)

            def mm_o():
                nc.tensor.matmul(state["yo"][:, off:512],
                                 vs[kk][:, 130 * p + 65:130 * p + 130],
                                 pt[:, 512 + off:1024],
                                 start=first, stop=last)

            fin = (lambda: emit_out(p, j, state)) if last else None
            return mm_e, mm_o, fin

        ptiles = {}
        states = {}

        def run_pair(p, stage_work, filler=None):
            """Emit the pair's attention as one flat pipeline in batches of
            two chunks: S/exp of batch b is emitted before PV of batch b-1
            (across q-block boundaries), so the in-order PE never stalls
            behind exp. stage_work (projection chains) is emitted at a
            q-block's first chunk; filler[i] work units are spliced in
            right after chunk i's S matmuls (PE food during exp waits)."""
            seq = [(j, kk) for j in range(NQB) for kk in range(4 * (j + 1))]
            batches = [seq[i:i + 2] for i in range(0, len(seq), 2)]
            filler = filler or {}

            def emit_pv(batch):
                parts = [attn_pv_part(p, pj, pkk,
                                      states.setdefault((p, pj), {}), ptiles)
                         for (pj, pkk) in batch]
                for e, o, _ in parts:
                    e()
                    o()
                for _, _, fin in parts:
                    if fin is not None:
                        fin()

            prev = None
            ci = 0
            for batch in batches:
                for (j, kk) in batch:
                    if kk == 0:
                        for w in stage_work.get(j, ()):
                            w()
                    attn_s_part(p, j, kk, ptiles)
                    for w in filler.get(ci, ()):
                        w()
                    ci += 1
                if prev is not None:
                    emit_pv(prev)
                prev = batch
            emit_pv(prev)

        # pair-0 stages: its own q/k projections + the first v tile of the
        # block; the remaining v tiles and pair 1's t-block-0 projections
        # are spliced between chunks as filler (q-blocks start at chunk
        # indices 0, 4, 12, 24; v(4j+i) must land ~i chunks in, before the
        # PV that consumes it)
        stage0 = {}
        for j in range(NQB):
            stage0[j] = [lambda j=j: qk_chain(0, 0, j),
                         lambda j=j: qk_chain(0, 1, j)]
            if j:
                stage0[j].append(lambda j=j: v_chain(4 * j))
        fill0 = {0: [lambda: v_chain(0)]}
        for j, base in enumerate((0, 4, 12, 24)):
            for i in (1, 2, 3):
                fill0.setdefault(base + i - 1, []).append(
                    lambda tt=4 * j + i: v_chain(tt))

        def funit(tb, ft_kind, half):
            return lambda: qk_chain_part(1, ft_kind, tb, half)

        for ci, (tb, ft_kind, half) in [
                (27, (0, 0, 0)), (29, (0, 0, 1)),
                (31, (0, 1, 0)), (33, (0, 1, 1))]:
            fill0.setdefault(ci, []).append(funit(tb, ft_kind, half))

        # pair-1 filler: its remaining projection chains in 4-matmul units,
        # spread across the chunks of the preceding q-block (each unit must
        # land before its stage starts: stages begin at chunks 4, 12, 24)
        fill1 = {}
        for ci, (tb, ft_kind, half) in [
                (0, (1, 0, 0)), (1, (1, 0, 1)), (2, (1, 1, 0)), (3, (1, 1, 1)),
                (5, (2, 0, 0)), (7, (2, 0, 1)), (9, (2, 1, 0)), (11, (2, 1, 1)),
                (13, (3, 0, 0)), (16, (3, 0, 1)), (19, (3, 1, 0)),
                (22, (3, 1, 1))]:
            fill1.setdefault(ci, []).append(funit(tb, ft_kind, half))

        run_pair(0, stage0, fill0)
        run_pair(1, {}, fill1)

    nc.compile()
    return nc


def _get_nc():
    global _NC
    if _NC is None:
        _NC = _build()
    return _NC


def _make_in_maps(x, W_attn):
    x = np.asarray(x, dtype=np.float32)
    W = np.asarray(W_attn, dtype=np.float32)
    wq, wk, wv = W[0:C], W[C:2 * C], W[2 * C:3 * C]
    bf = ml_dtypes.bfloat16
    in_maps = []
    for c in range(NCORES):
        b, g = c // 4, c % 4
        heads = [HPC * g + i for i in range(HPC)]
        xTb = np.ascontiguousarray(x[b].T).astype(bf)
        qrows = np.concatenate([wq[D * h:D * h + D] for h in heads], axis=0)
        krows = np.concatenate([wk[D * h:D * h + D] for h in heads], axis=0)
        wqk_np = np.concatenate([qrows, krows], 0).T  # [C, 512]
        wv_np = np.zeros((C, HPC * 65), np.float32)
        for i, h in enumerate(heads):
            wv_np[:, 65 * i:65 * i + D] = wv[D * h:D * h + D].T
        # pre-tile on the partition axis so each weight loads in O(1)
        # dma_starts: wqk becomes [p, ft*1024 + c*128 + f] (ft-major, so
        # the first chain's q weights are a small leading transfer), wv
        # becomes [p, c*260 + f]
        wqk_t = np.ascontiguousarray(
            wqk_np.reshape(8, 128, 4, 128).transpose(1, 2, 0, 3)
            .reshape(128, -1))
        wv_t = np.ascontiguousarray(
            wv_np.reshape(8, 128, 260).transpose(1, 0, 2).reshape(128, -1))
        in_maps.append({"xT": xTb, "wqk": wqk_t.astype(bf),
                        "wv": wv_t.astype(bf)})
    return in_maps


def _execute(in_maps, trace=False):
    return run_bass_kernel_spmd(_get_nc(), in_maps,
                                core_ids=list(range(NCORES)), trace=trace)


def _assemble(results):
    y = np.empty((B, T, C), np.float32)
    for c in range(NCORES):
        b, g = c // 4, c % 4
        # [HPC, 65, T] bf16; row 64 = softmax denominator
        yc = results[c]["y"].astype(np.float32)
        for i in range(HPC):
            h = HPC * g + i
            y[b, :, D * h:D * h + D] = (yc[i, 0:64] / yc[i, 64:65]).T
    return y


def kernel(x, W_attn):
    res = _execute(_make_in_maps(x, W_attn), trace=False)
    return _assemble(res.results)


# revision 36
# speedup vs baseline: 1.0260x; 1.0150x over previous
"""Causal self-attention (B=2, T=2048, C=1024, H=16) on 8 TRN2 NeuronCores.

Sharding: core c handles batch b = c//4 and heads 4*(c%4) .. 4*(c%4)+3
(data-parallel over B, tensor-parallel over heads; full K/V for its heads
is computed locally from the core's QKV projection slice).

Optimizations over the f32r baseline (185us -> ~126us):
  - all matmul operands bf16 (separate LDWEIGHTS with FWL + pull-ahead
    instead of f32r's self-loading weight path; no f32r N<256 4x penalty;
    half the DMA bytes); PSUM accumulation stays f32
  - the two K=64 S-matmuls of a head pair land in disjoint PE row groups
    (auto tile_position from base partitions 0/64) and execute
    concurrently (second MM ends ~3ns after the first)
  - exp only over causally-valid columns; causal mask shrunk to the one
    128-column staircase band per diagonal tile (PV skips dead columns,
    so only the band needs zeroing) -> ~4x less gpsimd and a shorter
    exp->mask->PV critical path
  - softmax division done on the host during unshard (row 64 of each
    head's output carries the denominators); kills the on-device
    reciprocal/broadcast/multiply epilogue and its serial tail
  - startup was DMA descriptor-gen bound (~0.65us per dma_start,
    serialized per HWDGE ring): weights arrive pre-tiled so all input
    lands in 9 dma_starts issued in strict need-order, and the HAM
    warm-up bridges until the first chain's data arrives
  - projection chains are spliced between attention chunks in 4-matmul
    units so the in-order PE always has ready work while ScalarE grinds
    exp (the attention steady state is exp-bound at ~1.0us/chunk)

Per-core dataflow:
  - host passes xT = x[b].T [C,T] bf16, wqk [128, ft*1024+c*128+f] bf16
    (ft-major pre-tiled q/k weights), wv [128, c*260+f] bf16 (per head a
    65-wide block whose last column is zero)
  - qT/kT [64,T] per head via projection matmuls (contraction c on
    partitions), PSUM f32, cast to bf16 on the PSUM->SBUF copy
  - v [t,260] bf16 with a ones column appended per head (65th of a block)
  - head pairs (2p, 2p+1) share S^T tiles: s [k=128, 1024] = [S_even|S_odd]
    f32 PSUM, exp on ScalarE (scale=1/8 fused) -> bf16 pt
  - y^T [65, 512] += V'.T @ P^T accumulated over k-tiles; row 64 = softmax
    denominators (from the ones column)
  - y^T copied PSUM->SBUF bf16 (vector/scalar alternating), DMA'd out;
    host divides rows 0:64 by row 64, transposes, concats heads.
"""

import os
import sys
import types
import numpy as np
import ml_dtypes

import concourse.bass as bass
import concourse.mybir as mybir
import concourse.tile as tile
from concourse import bacc
from concourse.bass_utils import run_bass_kernel_spmd

B, T, C, H = 2, 2048, 1024, 16
D = 64
NCORES = 8
HPC = 4          # heads per core
NQB = 4          # q blocks of 512
QB = 512
F32 = mybir.dt.float32
BF16 = mybir.dt.bfloat16
EXP = mybir.ActivationFunctionType.Exp
IS_GE = mybir.AluOpType.is_ge


def _install_profhook():
    """Register the NTFF profile hook shim so BASS_TRACE=1 works; harmless
    no-op (graceful trace skip) when the axon .so lacks profiling."""
    if "antenv.axon_hooks" not in sys.modules:
        mod = types.ModuleType("antenv.axon_hooks")
        mod._hook = None
        mod.set_axon_ntff_profile_hook = lambda h: setattr(mod, "_hook", h)
        mod.get_axon_ntff_profile_hook = lambda: mod._hook
        sys.modules["antenv.axon_hooks"] = mod
        try:
            import antenv
            antenv.axon_hooks = mod
        except ImportError:
            pass
    try:
        from trn_agent_boot.trn_boot import _ntff_profile_via_ctypes
        sys.modules["antenv.axon_hooks"].set_axon_ntff_profile_hook(
            _ntff_profile_via_ctypes("/opt/axon/libaxon_pjrt.so")
        )
        import concourse.bass_utils as bu
        bu.upload_artifacts = lambda tmpdir: tmpdir
    except Exception:
        pass


_install_profhook()

_NC = None


def _build():
    nc = bacc.Bacc("TRN2", target_bir_lowering=False, debug=False,
                   num_devices=NCORES)
    # weights arrive pre-tiled on the partition axis (row p holds c-tile
    # slice [c*128+p, :] at columns c*W..) so each loads as ONE dma_start:
    # descriptor generation is ~0.65us per dma_start regardless of size,
    # and the startup was descriptor-gen bound
    xT_d = nc.declare_dram_parameter("xT", [C, T], BF16, isOutput=False)
    wqk_d = nc.declare_dram_parameter("wqk", [128, 4 * 8 * 128], BF16,
                                      isOutput=False)
    wv_d = nc.declare_dram_parameter("wv", [128, 8 * 260], BF16,
                                     isOutput=False)
    y_d = nc.declare_dram_parameter("y", [HPC, 65, T], BF16, isOutput=True)

    from contextlib import ExitStack
    with tile.TileContext(nc) as tc, ExitStack() as ctx:
        sb = ctx.enter_context(tc.tile_pool(name="sb", bufs=1))
        pp = ctx.enter_context(tc.tile_pool(name="pp", bufs=8))
        yp = ctx.enter_context(tc.tile_pool(name="yp", bufs=3))
        psp = ctx.enter_context(tc.tile_pool(name="psp", bufs=2, space="PSUM"))
        pss = ctx.enter_context(tc.tile_pool(name="pss", bufs=2, space="PSUM"))
        psy = ctx.enter_context(tc.tile_pool(name="psy", bufs=1, space="PSUM"))

        # x^T merged per t-block: xall[tb][p, c*512+f] = x[b].T[c*128+p,
        # tb*512+f] — each t-block loads as ONE dma_start (startup is both
        # descriptor-gen and bandwidth bound, so fewest gens in strict
        # need-order wins). wqk is ft-major so the first q-chain's weights
        # are their own small transfer.
        xall = [sb.tile([128, 8 * 512], BF16, name=f"xall{tb}")
                for tb in range(4)]
        wqks_all = sb.tile([128, 4 * 8 * 128], BF16, name="wqks")
        wvs_all = sb.tile([128, 8 * 260], BF16, name="wvs")
        qs = [[sb.tile([128, 512], BF16, name=f"q{p}_{tb}") for tb in range(4)]
              for p in range(2)]
        ks = [[sb.tile([128, 512], BF16, name=f"k{p}_{tb}") for tb in range(4)]
              for p in range(2)]
        vs = [sb.tile([128, 260], BF16, name=f"v_{t}") for t in range(16)]
        ones2 = sb.tile([128, 4], F32, name="ones2")
        nc.gpsimd.memset(ones2[:], 1.0)

        def xslice(tb, lo=0, hi=512):
            """AP slice of x^T covering t-block tb, contraction tile c."""
            return lambda c: xall[tb][:, c * 512 + lo:c * 512 + hi]

        # warm-up: keep the PE's HAM activity monitor busy while the input
        # DMAs land, so real matmuls start at 2.4 GHz instead of 1.2 GHz.
        # The operand memset rides the vector engine (gpsimd wakes ~3 us
        # later), and 40 matmuls (~17 cold then warm, ~6.3 us) bridge until the first
        # projection chain's inputs have arrived.
        wup = sb.tile([128, 256], BF16, name="wup")
        nc.vector.memset(wup[:], 0.5)
        wups = psp.tile([128, 256], F32, name="wups", tag="pmm")
        for _ in range(40):
            nc.tensor.matmul(wups[:], wup[:, 0:128], wup[:], start=True,
                             stop=True)

        # 8 dma_starts in strict need-order: q-pair0 weights (0.25MB), x
        # t-block 0 (1MB), k-pair0 weights, wv, pair-1 qk weights (one
        # strided transfer), then x t-blocks 1..3
        xT3 = xT_d.ap().rearrange("(c p) t -> p c t", c=8)  # [128, 8, 2048]

        def dma_xall(tb, c0=0, c1=8):
            nc.sync.dma_start(
                xall[tb][:, c0 * 512:c1 * 512].rearrange(
                    "p (c f) -> p c f", c=c1 - c0),
                xT3[:, c0:c1, tb * 512:(tb + 1) * 512])

        nc.sync.dma_start(wqks_all[:, 0:1024], wqk_d.ap()[:, 0:1024])
        # t-block 0 in halves so the first chain's c=0..3 matmuls can
        # start while c=4..7 is still in flight
        dma_xall(0, 0, 4)
        dma_xall(0, 4, 8)
        nc.sync.dma_start(wqks_all[:, 2048:3072], wqk_d.ap()[:, 2048:3072])
        nc.sync.dma_start(wvs_all[:], wv_d.ap()[:, :])
        nc.sync.dma_start(
            wqks_all[:].rearrange("p (b x) -> p b x", b=2)[:, :, 1024:2048],
            wqk_d.ap().rearrange("p (b x) -> p b x", b=2)[:, :, 1024:2048])
        for tb in (1, 2, 3):
            dma_xall(tb)

        qk_cache = {}

        def qk_chain_part(p, ft_kind, tb, half):
            """Half of one projection chain (4 accumulating matmuls); the
            second half finishes the accumulation and casts PSUM->SBUF."""
            ft = p if ft_kind == 0 else 2 + p
            key = (p, ft_kind, tb)
            if half == 0:
                qk_cache[key] = psp.tile([128, 512], F32,
                                         name=f"pqk{p}_{ft}_{tb}", tag="pmm")
            mm = qk_cache[key]
            xs = xslice(tb)
            for c in range(4 * half, 4 * half + 4):
                nc.tensor.matmul(mm[:],
                                 wqks_all[:, ft * 1024 + c * 128:
                                          ft * 1024 + (c + 1) * 128],
                                 xs(c),
                                 start=(c == 0), stop=(c == 7))
            if half == 1:
                dst = (qs if ft_kind == 0 else ks)[p][tb]
                nc.vector.tensor_copy(dst[:], mm[:])
                del qk_cache[key]

        def qk_chain(p, ft_kind, tb):
            qk_chain_part(p, ft_kind, tb, 0)
            qk_chain_part(p, ft_kind, tb, 1)

        def v_chain(tt):
            """Combined v projection for one t-tile (all 4 heads, N=260)."""
            tb, sub = tt // 4, tt % 4
            mmv = psp.tile([128, 260], F32, name=f"pv{tt}", tag="pmm")
            xs = xslice(tb, sub * 128, (sub + 1) * 128)
            for c in range(8):
                nc.tensor.matmul(mmv[:], xs(c),
                                 wvs_all[:, c * 260:(c + 1) * 260],
                                 start=(c == 0), stop=(c == 7))
            nc.vector.tensor_copy(vs[tt][:], mmv[:])
            nc.vector.tensor_copy(vs[tt][:, 64:260:65], ones2[:])

        def attn_s_part(p, j, kk, ptiles):
            """S matmuls + exp + causal mask for chunk (p, j, kk).

            Diagonal k-tiles only have valid scores for q >= k, i.e. local
            q >= off = 128*(kk-4j); matmuls and exp skip the dead columns
            (PV skips them too, so they can hold stale garbage), and only
            the 128-wide staircase band [off, off+128) needs masking."""
            off = max(0, 128 * (kk - 4 * j))
            s = pss.tile([128, 1024], F32, name=f"s{p}_{j}_{kk}", tag="s")
            ktb, ksub = kk // 4, (kk % 4) * 128
            nc.tensor.matmul(s[:, off:512],
                             ks[p][ktb][0:64, ksub:ksub + 128],
                             qs[p][j][0:64, off:512],
                             start=True, stop=True)
            nc.tensor.matmul(s[:, 512 + off:1024],
                             ks[p][ktb][64:128, ksub:ksub + 128],
                             qs[p][j][64:128, off:512],
                             start=True, stop=True)
            pt = pp.tile([128, 1024], BF16, name=f"pt{p}_{j}_{kk}", tag="pt")
            if off:
                nc.scalar.activation(
                    pt[:].rearrange("p (b q) -> p b q", b=2)[:, :, off:512],
                    s[:].rearrange("p (b q) -> p b q", b=2)[:, :, off:512],
                    EXP, scale=0.125)
            else:
                nc.scalar.activation(pt[:], s[:], EXP, scale=0.125)
            if kk >= 4 * j:
                # zero P where q < k; only the staircase band straddles the
                # diagonal (cols [off, off+128) of both head halves); the
                # predicate reduces to local_q >= key_partition (base=0)
                band = pt[:].rearrange("p (b q) -> p b q", b=2)[
                    :, :, off:off + 128]
                nc.gpsimd.affine_select(
                    band, band,
                    pattern=[[0, 2], [1, 128]],
                    compare_op=IS_GE, fill=0.0,
                    base=0,
                    channel_multiplier=-1)
            ptiles[(j, kk)] = pt

        def emit_out(p, j, state):
            """Copy the finished y^T accumulators (incl. denominator row 64)
            PSUM->SBUF as bf16 and DMA out in 4 row-chunks (spread across
            DMA queues); host does the divide. The very last output's
            second copy rides ScalarE (idle by then) so the two tail
            copies run in parallel; midstream both stay off ScalarE,
            which is the attention bottleneck (gpsimd cannot read PSUM)."""
            for h01, key in ((0, "ye"), (1, "yo")):
                ysb = yp.tile([65, 512], BF16,
                              name=f"ysb{p}_{j}_{h01}", tag="ysb")
                last = h01 == 1 and p == 1 and j == NQB - 1
                if last:
                    nc.scalar.copy(ysb[:], state[key][:])
                else:
                    nc.vector.tensor_copy(ysb[:], state[key][:])
                # one dma_start per head tile (the HW splits it across all
                # 16 SDMA engines); the final tile rides the scalar HWDGE
                # ring so the two tail DMAs drain in parallel
                eng = nc.scalar if last else nc.sync
                eng.dma_start(
                    y_d.ap()[2 * p + h01, :, j * 512:(j + 1) * 512], ysb[:])

        def attn_pv_part(p, j, kk, state, ptiles):
            """PV-accumulation closures for chunk (p, j, kk): one matmul per
            head half (so the caller can group same-PSUM-bank matmuls), plus
            a trailing output closure on the q-block's last k-tile."""
            nkt = 4 * (j + 1)
            if kk == 0:
                state["ye"] = psy.tile([65, 512], F32,
                                       name=f"ye{p}_{j}", tag="ye")
                state["yo"] = psy.tile([65, 512], F32,
                                       name=f"yo{p}_{j}", tag="yo")
            pt = ptiles.pop((j, kk))
            first, last = (kk == 0), (kk == nkt - 1)
            # skip columns where P is all-zero (above the causal diagonal);
            # their y contribution is zero and PSUM keeps the prior partials
            off = 0 if first else max(0, 128 * (kk - 4 * j))

            def mm_e():
                nc.tensor.matmul(state["ye"][:, off:512],
                                 vs[kk][:, 130 * p:130 * p + 65],
                                 pt[:, off:512],
                                 start=first, stop=last)

            def mm_o():
                nc.tensor.matmul(state["yo"][:, off:512],
                                 vs[kk][:, 130 * p + 65:130 * p + 130],
                                 pt[:, 512 + off:1024],
                                 start=first, stop=last)

            fin = (lambda: emit_out(p, j, state)) if last else None
            return mm_e, mm_o, fin

        ptiles = {}
        states = {}

        def run_pair(p, stage_work, filler=None):
            """Emit the pair's attention as one flat pipeline in batches of
            two chunks: S/exp of batch b is emitted before PV of batch b-1
            (across q-block boundaries), so the in-order PE never stalls
            behind exp. stage_work (projection chains) is emitted at a
            q-block's first chunk; filler[i] work units are spliced in
            right after chunk i's S matmuls (PE food during exp waits)."""
            seq = [(j, kk) for j in range(NQB) for kk in range(4 * (j + 1))]
            batches = [seq[i:i + 2] for i in range(0, len(seq), 2)]
            filler = filler or {}

            def emit_pv(batch):
                parts = [attn_pv_part(p, pj, pkk,
                                      states.setdefault((p, pj), {}), ptiles)
                         for (pj, pkk) in batch]
                for e, o, _ in parts:
                    e()
                    o()
                for _, _, fin in parts:
                    if fin is not None:
                        fin()

            prev = None
            ci = 0
            for batch in batches:
                # emit both chunks' K=64 S-pairs back-to-back, THEN the
                # K=128 fillers: a filler between the S-pairs would cost
                # two extra 64<->128 PE array reconfigurations (~100-200ns
                # each) per batch. Stage work (K=128) stays before the
                # S-group, adjacent to the previous batch's K=128 PVs.
                fls = []
                for (j, kk) in batch:
                    if kk == 0:
                        for w in stage_work.get(j, ()):
                            w()
                    attn_s_part(p, j, kk, ptiles)
                    fls += filler.get(ci, ())
                    ci += 1
                for w in fls:
                    w()
                if prev is not None:
                    emit_pv(prev)
                prev = batch
            emit_pv(prev)

        # pair-0 stages: its own q/k projections + the first v tile of the
        # block; the remaining v tiles and pair 1's t-block-0 projections
        # are spliced between chunks as filler (q-blocks start at chunk
        # indices 0, 4, 12, 24; v(4j+i) must land ~i chunks in, before the
        # PV that consumes it)
        stage0 = {}
        for j in range(NQB):
            stage0[j] = [lambda j=j: qk_chain(0, 0, j),
                         lambda j=j: qk_chain(0, 1, j)]
            if j:
                stage0[j].append(lambda j=j: v_chain(4 * j))
        fill0 = {0: [lambda: v_chain(0)]}
        for j, base in enumerate((0, 4, 12, 24)):
            for i in (1, 2, 3):
                fill0.setdefault(base + i - 1, []).append(
                    lambda tt=4 * j + i: v_chain(tt))

        def funit(tb, ft_kind, half):
            return lambda: qk_chain_part(1, ft_kind, tb, half)

        for ci, (tb, ft_kind, half) in [
                (27, (0, 0, 0)), (29, (0, 0, 1)),
                (31, (0, 1, 0)), (33, (0, 1, 1))]:
            fill0.setdefault(ci, []).append(funit(tb, ft_kind, half))

        # pair-1 filler: its remaining projection chains in 4-matmul units,
        # spread across the chunks of the preceding q-block (each unit must
        # land before its stage starts: stages begin at chunks 4, 12, 24)
        fill1 = {}
        for ci, (tb, ft_kind, half) in [
                (0, (1, 0, 0)), (1, (1, 0, 1)), (2, (1, 1, 0)), (3, (1, 1, 1)),
                (5, (2, 0, 0)), (7, (2, 0, 1)), (9, (2, 1, 0)), (11, (2, 1, 1)),
                (13, (3, 0, 0)), (16, (3, 0, 1)), (19, (3, 1, 0)),
                (22, (3, 1, 1))]:
            fill1.setdefault(ci, []).append(funit(tb, ft_kind, half))

        run_pair(0, stage0, fill0)
        run_pair(1, {}, fill1)

    nc.compile()
    return nc


def _get_nc():
    global _NC
    if _NC is None:
        _NC = _build()
    return _NC


def _make_in_maps(x, W_attn):
    x = np.asarray(x, dtype=np.float32)
    W = np.asarray(W_attn, dtype=np.float32)
    wq, wk, wv = W[0:C], W[C:2 * C], W[2 * C:3 * C]
    bf = ml_dtypes.bfloat16
    in_maps = []
    for c in range(NCORES):
        b, g = c // 4, c % 4
        heads = [HPC * g + i for i in range(HPC)]
        xTb = np.ascontiguousarray(x[b].T).astype(bf)
        qrows = np.concatenate([wq[D * h:D * h + D] for h in heads], axis=0)
        krows = np.concatenate([wk[D * h:D * h + D] for h in heads], axis=0)
        wqk_np = np.concatenate([qrows, krows], 0).T  # [C, 512]
        wv_np = np.zeros((C, HPC * 65), np.float32)
        for i, h in enumerate(heads):
            wv_np[:, 65 * i:65 * i + D] = wv[D * h:D * h + D].T
        # pre-tile on the partition axis so each weight loads in O(1)
        # dma_starts: wqk becomes [p, ft*1024 + c*128 + f] (ft-major, so
        # the first chain's q weights are a small leading transfer), wv
        # becomes [p, c*260 + f]
        wqk_t = np.ascontiguousarray(
            wqk_np.reshape(8, 128, 4, 128).transpose(1, 2, 0, 3)
            .reshape(128, -1))
        wv_t = np.ascontiguousarray(
            wv_np.reshape(8, 128, 260).transpose(1, 0, 2).reshape(128, -1))
        in_maps.append({"xT": xTb, "wqk": wqk_t.astype(bf),
                        "wv": wv_t.astype(bf)})
    return in_maps


def _execute(in_maps, trace=False):
    return run_bass_kernel_spmd(_get_nc(), in_maps,
                                core_ids=list(range(NCORES)), trace=trace)


def _assemble(results):
    y = np.empty((B, T, C), np.float32)
    for c in range(NCORES):
        b, g = c // 4, c % 4
        # [HPC, 65, T] bf16; row 64 = softmax denominator
        yc = results[c]["y"].astype(np.float32)
        for i in range(HPC):
            h = HPC * g + i
            y[b, :, D * h:D * h + D] = (yc[i, 0:64] / yc[i, 64:65]).T
    return y


def kernel(x, W_attn):
    res = _execute(_make_in_maps(x, W_attn), trace=False)
    return _assemble(res.results)


# revision 37
# speedup vs baseline: 1.0301x; 1.0039x over previous
"""Causal self-attention (B=2, T=2048, C=1024, H=16) on 8 TRN2 NeuronCores.

Sharding: core c handles batch b = c//4 and heads 4*(c%4) .. 4*(c%4)+3
(data-parallel over B, tensor-parallel over heads; full K/V for its heads
is computed locally from the core's QKV projection slice).

Optimizations over the f32r baseline (185us -> ~126us):
  - all matmul operands bf16 (separate LDWEIGHTS with FWL + pull-ahead
    instead of f32r's self-loading weight path; no f32r N<256 4x penalty;
    half the DMA bytes); PSUM accumulation stays f32
  - the two K=64 S-matmuls of a head pair land in disjoint PE row groups
    (auto tile_position from base partitions 0/64) and execute
    concurrently (second MM ends ~3ns after the first)
  - exp only over causally-valid columns; causal mask shrunk to the one
    128-column staircase band per diagonal tile (PV skips dead columns,
    so only the band needs zeroing) -> ~4x less gpsimd and a shorter
    exp->mask->PV critical path
  - softmax division done on the host during unshard (row 64 of each
    head's output carries the denominators); kills the on-device
    reciprocal/broadcast/multiply epilogue and its serial tail
  - startup was DMA descriptor-gen bound (~0.65us per dma_start,
    serialized per HWDGE ring): weights arrive pre-tiled so all input
    lands in 9 dma_starts issued in strict need-order, and the HAM
    warm-up bridges until the first chain's data arrives
  - projection chains are spliced between attention chunks in 4-matmul
    units so the in-order PE always has ready work while ScalarE grinds
    exp; within a batch they are grouped AFTER both K=64 S-pairs, since
    a K=128 matmul between them costs two extra 64<->128 PE array
    reconfigurations (~100-200ns each)

Per-core dataflow:
  - host passes xT = x[b].T [C,T] bf16, wqk [128, ft*1024+c*128+f] bf16
    (ft-major pre-tiled q/k weights), wv [128, c*260+f] bf16 (per head a
    65-wide block whose last column is zero)
  - qT/kT [64,T] per head via projection matmuls (contraction c on
    partitions), PSUM f32, cast to bf16 on the PSUM->SBUF copy
  - v [t,260] bf16 with a ones column appended per head (65th of a block)
  - head pairs (2p, 2p+1) share S^T tiles: s [k=128, 1024] = [S_even|S_odd]
    f32 PSUM, exp on ScalarE (scale=1/8 fused) -> bf16 pt
  - y^T [65, 512] += V'.T @ P^T accumulated over k-tiles; row 64 = softmax
    denominators (from the ones column)
  - y^T copied PSUM->SBUF bf16 (vector/scalar alternating), DMA'd out;
    host divides rows 0:64 by row 64, transposes, concats heads.
"""

import os
import sys
import types
import numpy as np
import ml_dtypes

import concourse.bass as bass
import concourse.mybir as mybir
import concourse.tile as tile
from concourse import bacc
from concourse.bass_utils import run_bass_kernel_spmd

B, T, C, H = 2, 2048, 1024, 16
D = 64
NCORES = 8
HPC = 4          # heads per core
NQB = 4          # q blocks of 512
QB = 512
F32 = mybir.dt.float32
BF16 = mybir.dt.bfloat16
EXP = mybir.ActivationFunctionType.Exp
IS_GE = mybir.AluOpType.is_ge


def _install_profhook():
    """Register the NTFF profile hook shim so BASS_TRACE=1 works; harmless
    no-op (graceful trace skip) when the axon .so lacks profiling."""
    if "antenv.axon_hooks" not in sys.modules:
        mod = types.ModuleType("antenv.axon_hooks")
        mod._hook = None
        mod.set_axon_ntff_profile_hook = lambda h: setattr(mod, "_hook", h)
        mod.get_axon_ntff_profile_hook = lambda: mod._hook
        sys.modules["antenv.axon_hooks"] = mod
        try:
            import antenv
            antenv.axon_hooks = mod
        except ImportError:
            pass
    try:
        from trn_agent_boot.trn_boot import _ntff_profile_via_ctypes
        sys.modules["antenv.axon_hooks"].set_axon_ntff_profile_hook(
            _ntff_profile_via_ctypes("/opt/axon/libaxon_pjrt.so")
        )
        import concourse.bass_utils as bu
        bu.upload_artifacts = lambda tmpdir: tmpdir
    except Exception:
        pass


_install_profhook()

_NC = None


def _build():
    nc = bacc.Bacc("TRN2", target_bir_lowering=False, debug=False,
                   num_devices=NCORES)
    # weights arrive pre-tiled on the partition axis (row p holds c-tile
    # slice [c*128+p, :] at columns c*W..) so each loads as ONE dma_start:
    # descriptor generation is ~0.65us per dma_start regardless of size,
    # and the startup was descriptor-gen bound
    xT_d = nc.declare_dram_parameter("xT", [C, T], BF16, isOutput=False)
    wqk_d = nc.declare_dram_parameter("wqk", [128, 4 * 8 * 128], BF16,
                                      isOutput=False)
    wv_d = nc.declare_dram_parameter("wv", [128, 8 * 260], BF16,
                                     isOutput=False)
    y_d = nc.declare_dram_parameter("y", [HPC, 65, T], BF16, isOutput=True)

    from contextlib import ExitStack
    with tile.TileContext(nc) as tc, ExitStack() as ctx:
        sb = ctx.enter_context(tc.tile_pool(name="sb", bufs=1))
        pp = ctx.enter_context(tc.tile_pool(name="pp", bufs=8))
        yp = ctx.enter_context(tc.tile_pool(name="yp", bufs=3))
        psp = ctx.enter_context(tc.tile_pool(name="psp", bufs=2, space="PSUM"))
        pss = ctx.enter_context(tc.tile_pool(name="pss", bufs=2, space="PSUM"))
        psy = ctx.enter_context(tc.tile_pool(name="psy", bufs=1, space="PSUM"))

        # x^T merged per t-block: xall[tb][p, c*512+f] = x[b].T[c*128+p,
        # tb*512+f] — each t-block loads as ONE dma_start (startup is both
        # descriptor-gen and bandwidth bound, so fewest gens in strict
        # need-order wins). wqk is ft-major so the first q-chain's weights
        # are their own small transfer.
        xall = [sb.tile([128, 8 * 512], BF16, name=f"xall{tb}")
                for tb in range(4)]
        wqks_all = sb.tile([128, 4 * 8 * 128], BF16, name="wqks")
        wvs_all = sb.tile([128, 8 * 260], BF16, name="wvs")
        qs = [[sb.tile([128, 512], BF16, name=f"q{p}_{tb}") for tb in range(4)]
              for p in range(2)]
        ks = [[sb.tile([128, 512], BF16, name=f"k{p}_{tb}") for tb in range(4)]
              for p in range(2)]
        vs = [sb.tile([128, 260], BF16, name=f"v_{t}") for t in range(16)]
        ones2 = sb.tile([128, 4], F32, name="ones2")
        nc.gpsimd.memset(ones2[:], 1.0)

        def xslice(tb, lo=0, hi=512):
            """AP slice of x^T covering t-block tb, contraction tile c."""
            return lambda c: xall[tb][:, c * 512 + lo:c * 512 + hi]

        # warm-up: keep the PE's HAM activity monitor busy while the input
        # DMAs land, so real matmuls start at 2.4 GHz instead of 1.2 GHz.
        # The operand memset rides the vector engine (gpsimd wakes ~3 us
        # later), and 40 matmuls (~17 cold then warm, ~6.3 us) bridge until the first
        # projection chain's inputs have arrived.
        wup = sb.tile([128, 256], BF16, name="wup")
        nc.vector.memset(wup[:], 0.5)
        wups = psp.tile([128, 256], F32, name="wups", tag="pmm")
        for _ in range(40):
            nc.tensor.matmul(wups[:], wup[:, 0:128], wup[:], start=True,
                             stop=True)

        # 8 dma_starts in strict need-order: q-pair0 weights (0.25MB), x
        # t-block 0 (1MB), k-pair0 weights, wv, pair-1 qk weights (one
        # strided transfer), then x t-blocks 1..3
        xT3 = xT_d.ap().rearrange("(c p) t -> p c t", c=8)  # [128, 8, 2048]

        def dma_xall(tb, c0=0, c1=8):
            nc.sync.dma_start(
                xall[tb][:, c0 * 512:c1 * 512].rearrange(
                    "p (c f) -> p c f", c=c1 - c0),
                xT3[:, c0:c1, tb * 512:(tb + 1) * 512])

        nc.sync.dma_start(wqks_all[:, 0:1024], wqk_d.ap()[:, 0:1024])
        # t-block 0 in halves so the first chain's c=0..3 matmuls can
        # start while c=4..7 is still in flight
        dma_xall(0, 0, 4)
        dma_xall(0, 4, 8)
        nc.sync.dma_start(wqks_all[:, 2048:3072], wqk_d.ap()[:, 2048:3072])
        nc.sync.dma_start(wvs_all[:], wv_d.ap()[:, :])
        nc.sync.dma_start(
            wqks_all[:].rearrange("p (b x) -> p b x", b=2)[:, :, 1024:2048],
            wqk_d.ap().rearrange("p (b x) -> p b x", b=2)[:, :, 1024:2048])
        for tb in (1, 2, 3):
            dma_xall(tb)

        qk_cache = {}

        def qk_chain_part(p, ft_kind, tb, half):
            """Half of one projection chain (4 accumulating matmuls); the
            second half finishes the accumulation and casts PSUM->SBUF."""
            ft = p if ft_kind == 0 else 2 + p
            key = (p, ft_kind, tb)
            if half == 0:
                qk_cache[key] = psp.tile([128, 512], F32,
                                         name=f"pqk{p}_{ft}_{tb}", tag="pmm")
            mm = qk_cache[key]
            xs = xslice(tb)
            for c in range(4 * half, 4 * half + 4):
                nc.tensor.matmul(mm[:],
                                 wqks_all[:, ft * 1024 + c * 128:
                                          ft * 1024 + (c + 1) * 128],
                                 xs(c),
                                 start=(c == 0), stop=(c == 7))
            if half == 1:
                dst = (qs if ft_kind == 0 else ks)[p][tb]
                nc.vector.tensor_copy(dst[:], mm[:])
                del qk_cache[key]

        def qk_chain(p, ft_kind, tb):
            qk_chain_part(p, ft_kind, tb, 0)
            qk_chain_part(p, ft_kind, tb, 1)

        def v_chain(tt):
            """Combined v projection for one t-tile (all 4 heads, N=260)."""
            tb, sub = tt // 4, tt % 4
            mmv = psp.tile([128, 260], F32, name=f"pv{tt}", tag="pmm")
            xs = xslice(tb, sub * 128, (sub + 1) * 128)
            for c in range(8):
                nc.tensor.matmul(mmv[:], xs(c),
                                 wvs_all[:, c * 260:(c + 1) * 260],
                                 start=(c == 0), stop=(c == 7))
            nc.vector.tensor_copy(vs[tt][:], mmv[:])
            nc.vector.tensor_copy(vs[tt][:, 64:260:65], ones2[:])

        def attn_s_part(p, j, kk, ptiles):
            """S matmuls + exp + causal mask for chunk (p, j, kk).

            Diagonal k-tiles only have valid scores for q >= k, i.e. local
            q >= off = 128*(kk-4j); matmuls and exp skip the dead columns
            (PV skips them too, so they can hold stale garbage), and only
            the 128-wide staircase band [off, off+128) needs masking."""
            off = max(0, 128 * (kk - 4 * j))
            s = pss.tile([128, 1024], F32, name=f"s{p}_{j}_{kk}", tag="s")
            ktb, ksub = kk // 4, (kk % 4) * 128
            nc.tensor.matmul(s[:, off:512],
                             ks[p][ktb][0:64, ksub:ksub + 128],
                             qs[p][j][0:64, off:512],
                             start=True, stop=True)
            nc.tensor.matmul(s[:, 512 + off:1024],
                             ks[p][ktb][64:128, ksub:ksub + 128],
                             qs[p][j][64:128, off:512],
                             start=True, stop=True)
            pt = pp.tile([128, 1024], BF16, name=f"pt{p}_{j}_{kk}", tag="pt")
            if off:
                nc.scalar.activation(
                    pt[:].rearrange("p (b q) -> p b q", b=2)[:, :, off:512],
                    s[:].rearrange("p (b q) -> p b q", b=2)[:, :, off:512],
                    EXP, scale=0.125)
            else:
                nc.scalar.activation(pt[:], s[:], EXP, scale=0.125)
            if kk >= 4 * j:
                # zero P where q < k; only the staircase band straddles the
                # diagonal (cols [off, off+128) of both head halves); the
                # predicate reduces to local_q >= key_partition (base=0)
                band = pt[:].rearrange("p (b q) -> p b q", b=2)[
                    :, :, off:off + 128]
                nc.gpsimd.affine_select(
                    band, band,
                    pattern=[[0, 2], [1, 128]],
                    compare_op=IS_GE, fill=0.0,
                    base=0,
                    channel_multiplier=-1)
            ptiles[(j, kk)] = pt

        def emit_out(p, j, state):
            """Copy the finished y^T accumulators (incl. denominator row 64)
            PSUM->SBUF as bf16 and DMA out in 4 row-chunks (spread across
            DMA queues); host does the divide. The very last output's
            second copy rides ScalarE (idle by then) so the two tail
            copies run in parallel; midstream both stay off ScalarE,
            which is the attention bottleneck (gpsimd cannot read PSUM)."""
            for h01, key in ((0, "ye"), (1, "yo")):
                ysb = yp.tile([65, 512], BF16,
                              name=f"ysb{p}_{j}_{h01}", tag="ysb")
                last = h01 == 1 and p == 1 and j == NQB - 1
                if last:
                    nc.scalar.copy(ysb[:], state[key][:])
                else:
                    nc.vector.tensor_copy(ysb[:], state[key][:])
                # one dma_start per head tile (the HW splits it across all
                # 16 SDMA engines); the final tile rides the scalar HWDGE
                # ring so the two tail DMAs drain in parallel
                eng = nc.scalar if last else nc.sync
                eng.dma_start(
                    y_d.ap()[2 * p + h01, :, j * 512:(j + 1) * 512], ysb[:])

        def attn_pv_part(p, j, kk, state, ptiles):
            """PV-accumulation closures for chunk (p, j, kk): one matmul per
            head half (so the caller can group same-PSUM-bank matmuls), plus
            a trailing output closure on the q-block's last k-tile."""
            nkt = 4 * (j + 1)
            if kk == 0:
                state["ye"] = psy.tile([65, 512], F32,
                                       name=f"ye{p}_{j}", tag="ye")
                state["yo"] = psy.tile([65, 512], F32,
                                       name=f"yo{p}_{j}", tag="yo")
            pt = ptiles.pop((j, kk))
            first, last = (kk == 0), (kk == nkt - 1)
            # skip columns where P is all-zero (above the causal diagonal);
            # their y contribution is zero and PSUM keeps the prior partials
            off = 0 if first else max(0, 128 * (kk - 4 * j))

            def mm_e():
                nc.tensor.matmul(state["ye"][:, off:512],
                                 vs[kk][:, 130 * p:130 * p + 65],
                                 pt[:, off:512],
                                 start=first, stop=last)

            def mm_o():
                nc.tensor.matmul(state["yo"][:, off:512],
                                 vs[kk][:, 130 * p + 65:130 * p + 130],
                                 pt[:, 512 + off:1024],
                                 start=first, stop=last)

            fin = (lambda: emit_out(p, j, state)) if last else None
            return mm_e, mm_o, fin

        ptiles = {}
        states = {}

        def run_pair(p, stage_work, filler=None):
            """Emit the pair's attention as one flat pipeline in batches of
            two chunks: S/exp of batch b is emitted before PV of batch b-1
            (across q-block boundaries), so the in-order PE never stalls
            behind exp. stage_work (projection chains) is emitted at a
            q-block's first chunk; filler[i] work units are spliced in
            right after chunk i's S matmuls (PE food during exp waits)."""
            seq = [(j, kk) for j in range(NQB) for kk in range(4 * (j + 1))]
            batches = [seq[i:i + 2] for i in range(0, len(seq), 2)]
            filler = filler or {}

            def emit_pv(batch):
                parts = [attn_pv_part(p, pj, pkk,
                                      states.setdefault((p, pj), {}), ptiles)
                         for (pj, pkk) in batch]
                for e, o, _ in parts:
                    e()
                    o()
                for _, _, fin in parts:
                    if fin is not None:
                        fin()

            prev = None
            ci = 0
            for batch in batches:
                # emit both chunks' K=64 S-pairs back-to-back, THEN the
                # K=128 fillers: a filler between the S-pairs would cost
                # two extra 64<->128 PE array reconfigurations (~100-200ns
                # each) per batch. Stage work (K=128) stays before the
                # S-group, adjacent to the previous batch's K=128 PVs.
                fls = []
                for (j, kk) in batch:
                    if kk == 0:
                        for w in stage_work.get(j, ()):
                            w()
                    attn_s_part(p, j, kk, ptiles)
                    fls += filler.get(ci, ())
                    ci += 1
                for w in fls:
                    w()
                if prev is not None:
                    emit_pv(prev)
                prev = batch
            emit_pv(prev)

        # pair-0 stages: its own q/k projections + the first v tile of the
        # block; the remaining v tiles and pair 1's t-block-0 projections
        # are spliced between chunks as filler (q-blocks start at chunk
        # indices 0, 4, 12, 24; v(4j+i) must land ~i chunks in, before the
        # PV that consumes it)
        stage0 = {}
        for j in range(NQB):
            stage0[j] = [lambda j=j: qk_chain(0, 0, j),
                         lambda j=j: qk_chain(0, 1, j)]
            if j:
                stage0[j].append(lambda j=j: v_chain(4 * j))
        fill0 = {0: [lambda: v_chain(0)]}
        for j, base in enumerate((0, 4, 12, 24)):
            for i in (1, 2, 3):
                fill0.setdefault(base + i - 1, []).append(
                    lambda tt=4 * j + i: v_chain(tt))

        def funit(tb, ft_kind, half):
            return lambda: qk_chain_part(1, ft_kind, tb, half)

        for ci, (tb, ft_kind, half) in [
                (27, (0, 0, 0)), (29, (0, 0, 1)),
                (31, (0, 1, 0)), (33, (0, 1, 1))]:
            fill0.setdefault(ci, []).append(funit(tb, ft_kind, half))

        # pair-1 filler: its remaining projection chains in 4-matmul units,
        # spread across the chunks of the preceding q-block (each unit must
        # land before its stage starts: stages begin at chunks 4, 12, 24)
        fill1 = {}
        for ci, (tb, ft_kind, half) in [
                (0, (1, 0, 0)), (1, (1, 0, 1)), (2, (1, 1, 0)), (3, (1, 1, 1)),
                (5, (2, 0, 0)), (7, (2, 0, 1)), (9, (2, 1, 0)), (11, (2, 1, 1)),
                (13, (3, 0, 0)), (16, (3, 0, 1)), (19, (3, 1, 0)),
                (22, (3, 1, 1))]:
            fill1.setdefault(ci, []).append(funit(tb, ft_kind, half))

        run_pair(0, stage0, fill0)
        run_pair(1, {}, fill1)

    nc.compile()
    return nc


def _get_nc():
    global _NC
    if _NC is None:
        _NC = _build()
    return _NC


def _make_in_maps(x, W_attn):
    x = np.asarray(x, dtype=np.float32)
    W = np.asarray(W_attn, dtype=np.float32)
    wq, wk, wv = W[0:C], W[C:2 * C], W[2 * C:3 * C]
    bf = ml_dtypes.bfloat16
    in_maps = []
    for c in range(NCORES):
        b, g = c // 4, c % 4
        heads = [HPC * g + i for i in range(HPC)]
        xTb = np.ascontiguousarray(x[b].T).astype(bf)
        qrows = np.concatenate([wq[D * h:D * h + D] for h in heads], axis=0)
        krows = np.concatenate([wk[D * h:D * h + D] for h in heads], axis=0)
        wqk_np = np.concatenate([qrows, krows], 0).T  # [C, 512]
        wv_np = np.zeros((C, HPC * 65), np.float32)
        for i, h in enumerate(heads):
            wv_np[:, 65 * i:65 * i + D] = wv[D * h:D * h + D].T
        # pre-tile on the partition axis so each weight loads in O(1)
        # dma_starts: wqk becomes [p, ft*1024 + c*128 + f] (ft-major, so
        # the first chain's q weights are a small leading transfer), wv
        # becomes [p, c*260 + f]
        wqk_t = np.ascontiguousarray(
            wqk_np.reshape(8, 128, 4, 128).transpose(1, 2, 0, 3)
            .reshape(128, -1))
        wv_t = np.ascontiguousarray(
            wv_np.reshape(8, 128, 260).transpose(1, 0, 2).reshape(128, -1))
        in_maps.append({"xT": xTb, "wqk": wqk_t.astype(bf),
                        "wv": wv_t.astype(bf)})
    return in_maps


def _execute(in_maps, trace=False):
    return run_bass_kernel_spmd(_get_nc(), in_maps,
                                core_ids=list(range(NCORES)), trace=trace)


def _assemble(results):
    y = np.empty((B, T, C), np.float32)
    for c in range(NCORES):
        b, g = c // 4, c % 4
        # [HPC, 65, T] bf16; row 64 = softmax denominator
        yc = results[c]["y"].astype(np.float32)
        for i in range(HPC):
            h = HPC * g + i
            y[b, :, D * h:D * h + D] = (yc[i, 0:64] / yc[i, 64:65]).T
    return y


def kernel(x, W_attn):
    res = _execute(_make_in_maps(x, W_attn), trace=False)
    return _assemble(res.results)


# revision 38
# speedup vs baseline: 1.0339x; 1.0037x over previous
"""Causal self-attention (B=2, T=2048, C=1024, H=16) on 8 TRN2 NeuronCores.

Sharding: core c handles batch b = c//4 and heads 4*(c%4) .. 4*(c%4)+3
(data-parallel over B, tensor-parallel over heads; full K/V for its heads
is computed locally from the core's QKV projection slice).

Optimizations over the f32r baseline (185us -> ~126us):
  - all matmul operands bf16 (separate LDWEIGHTS with FWL + pull-ahead
    instead of f32r's self-loading weight path; no f32r N<256 4x penalty;
    half the DMA bytes); PSUM accumulation stays f32
  - the two K=64 S-matmuls of a head pair land in disjoint PE row groups
    (auto tile_position from base partitions 0/64) and execute
    concurrently (second MM ends ~3ns after the first)
  - exp only over causally-valid columns; causal mask shrunk to the one
    128-column staircase band per diagonal tile (PV skips dead columns,
    so only the band needs zeroing) -> ~4x less gpsimd and a shorter
    exp->mask->PV critical path
  - softmax division done on the host during unshard (row 64 of each
    head's output carries the denominators); kills the on-device
    reciprocal/broadcast/multiply epilogue and its serial tail
  - startup was DMA descriptor-gen bound (~0.65us per dma_start,
    serialized per HWDGE ring): weights arrive pre-tiled so all input
    lands in 9 dma_starts issued in strict need-order, and the HAM
    warm-up bridges until the first chain's data arrives
  - projection chains are spliced between attention chunks in 4-matmul
    units so the in-order PE always has ready work while ScalarE grinds
    exp; within a batch they are grouped AFTER both K=64 S-pairs, since
    a K=128 matmul between them costs two extra 64<->128 PE array
    reconfigurations (~100-200ns each)

Per-core dataflow:
  - host passes xT = x[b].T [C,T] bf16, wqk [128, ft*1024+c*128+f] bf16
    (ft-major pre-tiled q/k weights), wv [128, c*260+f] bf16 (per head a
    65-wide block whose last column is zero)
  - qT/kT [64,T] per head via projection matmuls (contraction c on
    partitions), PSUM f32, cast to bf16 on the PSUM->SBUF copy
  - v [t,260] bf16 with a ones column appended per head (65th of a block)
  - head pairs (2p, 2p+1) share S^T tiles: s [k=128, 1024] = [S_even|S_odd]
    f32 PSUM, exp on ScalarE (scale=1/8 fused) -> bf16 pt
  - y^T [65, 512] += V'.T @ P^T accumulated over k-tiles; row 64 = softmax
    denominators (from the ones column)
  - y^T copied PSUM->SBUF bf16 (vector/scalar alternating), DMA'd out;
    host divides rows 0:64 by row 64, transposes, concats heads.
"""

import os
import sys
import types
import numpy as np
import ml_dtypes

import concourse.bass as bass
import concourse.mybir as mybir
import concourse.tile as tile
from concourse import bacc
from concourse.bass_utils import run_bass_kernel_spmd

B, T, C, H = 2, 2048, 1024, 16
D = 64
NCORES = 8
HPC = 4          # heads per core
NQB = 4          # q blocks of 512
QB = 512
F32 = mybir.dt.float32
BF16 = mybir.dt.bfloat16
EXP = mybir.ActivationFunctionType.Exp
IS_GE = mybir.AluOpType.is_ge


def _install_profhook():
    """Register the NTFF profile hook shim so BASS_TRACE=1 works; harmless
    no-op (graceful trace skip) when the axon .so lacks profiling."""
    if "antenv.axon_hooks" not in sys.modules:
        mod = types.ModuleType("antenv.axon_hooks")
        mod._hook = None
        mod.set_axon_ntff_profile_hook = lambda h: setattr(mod, "_hook", h)
        mod.get_axon_ntff_profile_hook = lambda: mod._hook
        sys.modules["antenv.axon_hooks"] = mod
        try:
            import antenv
            antenv.axon_hooks = mod
        except ImportError:
            pass
    try:
        from trn_agent_boot.trn_boot import _ntff_profile_via_ctypes
        sys.modules["antenv.axon_hooks"].set_axon_ntff_profile_hook(
            _ntff_profile_via_ctypes("/opt/axon/libaxon_pjrt.so")
        )
        import concourse.bass_utils as bu
        bu.upload_artifacts = lambda tmpdir: tmpdir
    except Exception:
        pass


_install_profhook()

_NC = None


def _build():
    nc = bacc.Bacc("TRN2", target_bir_lowering=False, debug=False,
                   num_devices=NCORES)
    # weights arrive pre-tiled on the partition axis (row p holds c-tile
    # slice [c*128+p, :] at columns c*W..) so each loads as ONE dma_start:
    # descriptor generation is ~0.65us per dma_start regardless of size,
    # and the startup was descriptor-gen bound
    xT_d = nc.declare_dram_parameter("xT", [C, T], BF16, isOutput=False)
    wqk_d = nc.declare_dram_parameter("wqk", [128, 4 * 8 * 128], BF16,
                                      isOutput=False)
    wv_d = nc.declare_dram_parameter("wv", [128, 8 * 260], BF16,
                                     isOutput=False)
    y_d = nc.declare_dram_parameter("y", [HPC, 65, T], BF16, isOutput=True)

    from contextlib import ExitStack
    with tile.TileContext(nc) as tc, ExitStack() as ctx:
        sb = ctx.enter_context(tc.tile_pool(name="sb", bufs=1))
        pp = ctx.enter_context(tc.tile_pool(name="pp", bufs=8))
        yp = ctx.enter_context(tc.tile_pool(name="yp", bufs=3))
        psp = ctx.enter_context(tc.tile_pool(name="psp", bufs=2, space="PSUM"))
        pss = ctx.enter_context(tc.tile_pool(name="pss", bufs=2, space="PSUM"))
        psy = ctx.enter_context(tc.tile_pool(name="psy", bufs=1, space="PSUM"))

        # x^T merged per t-block: xall[tb][p, c*512+f] = x[b].T[c*128+p,
        # tb*512+f] — each t-block loads as ONE dma_start (startup is both
        # descriptor-gen and bandwidth bound, so fewest gens in strict
        # need-order wins). wqk is ft-major so the first q-chain's weights
        # are their own small transfer.
        xall = [sb.tile([128, 8 * 512], BF16, name=f"xall{tb}")
                for tb in range(4)]
        wqks_all = sb.tile([128, 4 * 8 * 128], BF16, name="wqks")
        wvs_all = sb.tile([128, 8 * 260], BF16, name="wvs")
        qs = [[sb.tile([128, 512], BF16, name=f"q{p}_{tb}") for tb in range(4)]
              for p in range(2)]
        ks = [[sb.tile([128, 512], BF16, name=f"k{p}_{tb}") for tb in range(4)]
              for p in range(2)]
        vs = [sb.tile([128, 260], BF16, name=f"v_{t}") for t in range(16)]
        ones2 = sb.tile([128, 4], F32, name="ones2")
        nc.gpsimd.memset(ones2[:], 1.0)

        def xslice(tb, lo=0, hi=512):
            """AP slice of x^T covering t-block tb, contraction tile c."""
            return lambda c: xall[tb][:, c * 512 + lo:c * 512 + hi]

        # warm-up: keep the PE's HAM activity monitor busy while the input
        # DMAs land, so real matmuls start at 2.4 GHz instead of 1.2 GHz.
        # The operand memset rides the vector engine (gpsimd wakes ~3 us
        # later), and 40 matmuls (~17 cold then warm, ~6.3 us) bridge until the first
        # projection chain's inputs have arrived.
        wup = sb.tile([128, 256], BF16, name="wup")
        nc.vector.memset(wup[:], 0.5)
        wups = psp.tile([128, 256], F32, name="wups", tag="pmm")
        for _ in range(40):
            nc.tensor.matmul(wups[:], wup[:, 0:128], wup[:], start=True,
                             stop=True)

        # 8 dma_starts in strict need-order: q-pair0 weights (0.25MB), x
        # t-block 0 (1MB), k-pair0 weights, wv, pair-1 qk weights (one
        # strided transfer), then x t-blocks 1..3
        xT3 = xT_d.ap().rearrange("(c p) t -> p c t", c=8)  # [128, 8, 2048]

        def dma_xall(tb, c0=0, c1=8):
            nc.sync.dma_start(
                xall[tb][:, c0 * 512:c1 * 512].rearrange(
                    "p (c f) -> p c f", c=c1 - c0),
                xT3[:, c0:c1, tb * 512:(tb + 1) * 512])

        nc.sync.dma_start(wqks_all[:, 0:1024], wqk_d.ap()[:, 0:1024])
        # t-block 0 in halves so the first chain's c=0..3 matmuls can
        # start while c=4..7 is still in flight
        dma_xall(0, 0, 4)
        dma_xall(0, 4, 8)
        nc.sync.dma_start(wqks_all[:, 2048:3072], wqk_d.ap()[:, 2048:3072])
        nc.sync.dma_start(wvs_all[:], wv_d.ap()[:, :])
        nc.sync.dma_start(
            wqks_all[:].rearrange("p (b x) -> p b x", b=2)[:, :, 1024:2048],
            wqk_d.ap().rearrange("p (b x) -> p b x", b=2)[:, :, 1024:2048])
        for tb in (1, 2, 3):
            dma_xall(tb)

        qk_cache = {}

        def qk_chain_part(p, ft_kind, tb, half):
            """Half of one projection chain (4 accumulating matmuls); the
            second half finishes the accumulation and casts PSUM->SBUF."""
            ft = p if ft_kind == 0 else 2 + p
            key = (p, ft_kind, tb)
            if half == 0:
                qk_cache[key] = psp.tile([128, 512], F32,
                                         name=f"pqk{p}_{ft}_{tb}", tag="pmm")
            mm = qk_cache[key]
            xs = xslice(tb)
            for c in range(4 * half, 4 * half + 4):
                nc.tensor.matmul(mm[:],
                                 wqks_all[:, ft * 1024 + c * 128:
                                          ft * 1024 + (c + 1) * 128],
                                 xs(c),
                                 start=(c == 0), stop=(c == 7))
            if half == 1:
                dst = (qs if ft_kind == 0 else ks)[p][tb]
                if p == 0 and ft_kind == 1 and tb == 0:
                    # first k of the whole kernel: S(0,0) reads only
                    # cols 0:128, so give it its own small copy
                    nc.vector.tensor_copy(dst[:, 0:128], mm[:, 0:128])
                    nc.vector.tensor_copy(dst[:, 128:512], mm[:, 128:512])
                else:
                    nc.vector.tensor_copy(dst[:], mm[:])
                del qk_cache[key]

        def qk_chain(p, ft_kind, tb):
            qk_chain_part(p, ft_kind, tb, 0)
            qk_chain_part(p, ft_kind, tb, 1)

        def v_chain(tt):
            """Combined v projection for one t-tile (all 4 heads, N=260)."""
            tb, sub = tt // 4, tt % 4
            mmv = psp.tile([128, 260], F32, name=f"pv{tt}", tag="pmm")
            xs = xslice(tb, sub * 128, (sub + 1) * 128)
            for c in range(8):
                nc.tensor.matmul(mmv[:], xs(c),
                                 wvs_all[:, c * 260:(c + 1) * 260],
                                 start=(c == 0), stop=(c == 7))
            nc.vector.tensor_copy(vs[tt][:], mmv[:])
            nc.vector.tensor_copy(vs[tt][:, 64:260:65], ones2[:])

        def attn_s_part(p, j, kk, ptiles):
            """S matmuls + exp + causal mask for chunk (p, j, kk).

            Diagonal k-tiles only have valid scores for q >= k, i.e. local
            q >= off = 128*(kk-4j); matmuls and exp skip the dead columns
            (PV skips them too, so they can hold stale garbage), and only
            the 128-wide staircase band [off, off+128) needs masking."""
            off = max(0, 128 * (kk - 4 * j))
            s = pss.tile([128, 1024], F32, name=f"s{p}_{j}_{kk}", tag="s")
            ktb, ksub = kk // 4, (kk % 4) * 128
            nc.tensor.matmul(s[:, off:512],
                             ks[p][ktb][0:64, ksub:ksub + 128],
                             qs[p][j][0:64, off:512],
                             start=True, stop=True)
            nc.tensor.matmul(s[:, 512 + off:1024],
                             ks[p][ktb][64:128, ksub:ksub + 128],
                             qs[p][j][64:128, off:512],
                             start=True, stop=True)
            pt = pp.tile([128, 1024], BF16, name=f"pt{p}_{j}_{kk}", tag="pt")
            if off:
                nc.scalar.activation(
                    pt[:].rearrange("p (b q) -> p b q", b=2)[:, :, off:512],
                    s[:].rearrange("p (b q) -> p b q", b=2)[:, :, off:512],
                    EXP, scale=0.125)
            else:
                nc.scalar.activation(pt[:], s[:], EXP, scale=0.125)
            if kk >= 4 * j:
                # zero P where q < k; only the staircase band straddles the
                # diagonal (cols [off, off+128) of both head halves); the
                # predicate reduces to local_q >= key_partition (base=0)
                band = pt[:].rearrange("p (b q) -> p b q", b=2)[
                    :, :, off:off + 128]
                nc.gpsimd.affine_select(
                    band, band,
                    pattern=[[0, 2], [1, 128]],
                    compare_op=IS_GE, fill=0.0,
                    base=0,
                    channel_multiplier=-1)
            ptiles[(j, kk)] = pt

        def emit_out(p, j, state):
            """Copy the finished y^T accumulators (incl. denominator row 64)
            PSUM->SBUF as bf16 and DMA out in 4 row-chunks (spread across
            DMA queues); host does the divide. The very last output's
            second copy rides ScalarE (idle by then) so the two tail
            copies run in parallel; midstream both stay off ScalarE,
            which is the attention bottleneck (gpsimd cannot read PSUM)."""
            for h01, key in ((0, "ye"), (1, "yo")):
                ysb = yp.tile([65, 512], BF16,
                              name=f"ysb{p}_{j}_{h01}", tag="ysb")
                last = h01 == 1 and p == 1 and j == NQB - 1
                if last:
                    nc.scalar.copy(ysb[:], state[key][:])
                else:
                    nc.vector.tensor_copy(ysb[:], state[key][:])
                # one dma_start per head tile (the HW splits it across all
                # 16 SDMA engines); the final tile rides the scalar HWDGE
                # ring so the two tail DMAs drain in parallel
                eng = nc.scalar if last else nc.sync
                eng.dma_start(
                    y_d.ap()[2 * p + h01, :, j * 512:(j + 1) * 512], ysb[:])

        def attn_pv_part(p, j, kk, state, ptiles):
            """PV-accumulation closures for chunk (p, j, kk): one matmul per
            head half (so the caller can group same-PSUM-bank matmuls), plus
            a trailing output closure on the q-block's last k-tile."""
            nkt = 4 * (j + 1)
            if kk == 0:
                state["ye"] = psy.tile([65, 512], F32,
                                       name=f"ye{p}_{j}", tag="ye")
                state["yo"] = psy.tile([65, 512], F32,
                                       name=f"yo{p}_{j}", tag="yo")
            pt = ptiles.pop((j, kk))
            first, last = (kk == 0), (kk == nkt - 1)
            # skip columns where P is all-zero (above the causal diagonal);
            # their y contribution is zero and PSUM keeps the prior partials
            off = 0 if first else max(0, 128 * (kk - 4 * j))

            def mm_e():
                nc.tensor.matmul(state["ye"][:, off:512],
                                 vs[kk][:, 130 * p:130 * p + 65],
                                 pt[:, off:512],
                                 start=first, stop=last)

            def mm_o():
                nc.tensor.matmul(state["yo"][:, off:512],
                                 vs[kk][:, 130 * p + 65:130 * p + 130],
                                 pt[:, 512 + off:1024],
                                 start=first, stop=last)

            fin = (lambda: emit_out(p, j, state)) if last else None
            return mm_e, mm_o, fin

        ptiles = {}
        states = {}

        def run_pair(p, stage_work, filler=None):
            """Emit the pair's attention as one flat pipeline in batches of
            two chunks: S/exp of batch b is emitted before PV of batch b-1
            (across q-block boundaries), so the in-order PE never stalls
            behind exp. stage_work (projection chains) is emitted at a
            q-block's first chunk; filler[i] work units are spliced in
            right after chunk i's S matmuls (PE food during exp waits)."""
            seq = [(j, kk) for j in range(NQB) for kk in range(4 * (j + 1))]
            batches = [seq[i:i + 2] for i in range(0, len(seq), 2)]
            filler = filler or {}

            def emit_pv(batch):
                parts = [attn_pv_part(p, pj, pkk,
                                      states.setdefault((p, pj), {}), ptiles)
                         for (pj, pkk) in batch]
                for e, o, _ in parts:
                    e()
                    o()
                for _, _, fin in parts:
                    if fin is not None:
                        fin()

            prev = None
            ci = 0
            for batch in batches:
                # emit both chunks' K=64 S-pairs back-to-back, THEN the
                # K=128 fillers: a filler between the S-pairs would cost
                # two extra 64<->128 PE array reconfigurations (~100-200ns
                # each) per batch. Stage work (K=128) stays before the
                # S-group, adjacent to the previous batch's K=128 PVs.
                fls = []
                for (j, kk) in batch:
                    if kk == 0:
                        for w in stage_work.get(j, ()):
                            w()
                    attn_s_part(p, j, kk, ptiles)
                    fls += filler.get(ci, ())
                    ci += 1
                for w in fls:
                    w()
                if prev is not None:
                    emit_pv(prev)
                prev = batch
            emit_pv(prev)

        # pair-0 stages: its own q/k projections + the first v tile of the
        # block; the remaining v tiles and pair 1's t-block-0 projections
        # are spliced between chunks as filler (q-blocks start at chunk
        # indices 0, 4, 12, 24; v(4j+i) must land ~i chunks in, before the
        # PV that consumes it)
        stage0 = {}
        for j in range(NQB):
            stage0[j] = [lambda j=j: qk_chain(0, 0, j),
                         lambda j=j: qk_chain(0, 1, j)]
            if j:
                stage0[j].append(lambda j=j: v_chain(4 * j))
        fill0 = {0: [lambda: v_chain(0)]}
        for j, base in enumerate((0, 4, 12, 24)):
            for i in (1, 2, 3):
                fill0.setdefault(base + i - 1, []).append(
                    lambda tt=4 * j + i: v_chain(tt))

        def funit(tb, ft_kind, half):
            return lambda: qk_chain_part(1, ft_kind, tb, half)

        for ci, (tb, ft_kind, half) in [
                (27, (0, 0, 0)), (29, (0, 0, 1)),
                (31, (0, 1, 0)), (33, (0, 1, 1))]:
            fill0.setdefault(ci, []).append(funit(tb, ft_kind, half))

        # pair-1 filler: its remaining projection chains in 4-matmul units,
        # spread across the chunks of the preceding q-block (each unit must
        # land before its stage starts: stages begin at chunks 4, 12, 24)
        fill1 = {}
        for ci, (tb, ft_kind, half) in [
                (0, (1, 0, 0)), (1, (1, 0, 1)), (2, (1, 1, 0)), (3, (1, 1, 1)),
                (5, (2, 0, 0)), (7, (2, 0, 1)), (9, (2, 1, 0)), (11, (2, 1, 1)),
                (13, (3, 0, 0)), (16, (3, 0, 1)), (26, (3, 1, 0)),
                (30, (3, 1, 1))]:
            fill1.setdefault(ci, []).append(funit(tb, ft_kind, half))

        run_pair(0, stage0, fill0)
        run_pair(1, {}, fill1)

    nc.compile()
    return nc


def _get_nc():
    global _NC
    if _NC is None:
        _NC = _build()
    return _NC


def _make_in_maps(x, W_attn):
    x = np.asarray(x, dtype=np.float32)
    W = np.asarray(W_attn, dtype=np.float32)
    wq, wk, wv = W[0:C], W[C:2 * C], W[2 * C:3 * C]
    bf = ml_dtypes.bfloat16
    in_maps = []
    for c in range(NCORES):
        b, g = c // 4, c % 4
        heads = [HPC * g + i for i in range(HPC)]
        xTb = np.ascontiguousarray(x[b].T).astype(bf)
        qrows = np.concatenate([wq[D * h:D * h + D] for h in heads], axis=0)
        krows = np.concatenate([wk[D * h:D * h + D] for h in heads], axis=0)
        wqk_np = np.concatenate([qrows, krows], 0).T  # [C, 512]
        wv_np = np.zeros((C, HPC * 65), np.float32)
        for i, h in enumerate(heads):
            wv_np[:, 65 * i:65 * i + D] = wv[D * h:D * h + D].T
        # pre-tile on the partition axis so each weight loads in O(1)
        # dma_starts: wqk becomes [p, ft*1024 + c*128 + f] (ft-major, so
        # the first chain's q weights are a small leading transfer), wv
        # becomes [p, c*260 + f]
        wqk_t = np.ascontiguousarray(
            wqk_np.reshape(8, 128, 4, 128).transpose(1, 2, 0, 3)
            .reshape(128, -1))
        wv_t = np.ascontiguousarray(
            wv_np.reshape(8, 128, 260).transpose(1, 0, 2).reshape(128, -1))
        in_maps.append({"xT": xTb, "wqk": wqk_t.astype(bf),
                        "wv": wv_t.astype(bf)})
    return in_maps


def _execute(in_maps, trace=False):
    return run_bass_kernel_spmd(_get_nc(), in_maps,
                                core_ids=list(range(NCORES)), trace=trace)


def _assemble(results):
    y = np.empty((B, T, C), np.float32)
    for c in range(NCORES):
        b, g = c // 4, c % 4
        # [HPC, 65, T] bf16; row 64 = softmax denominator
        yc = results[c]["y"].astype(np.float32)
        for i in range(HPC):
            h = HPC * g + i
            y[b, :, D * h:D * h + D] = (yc[i, 0:64] / yc[i, 64:65]).T
    return y


def kernel(x, W_attn):
    res = _execute(_make_in_maps(x, W_attn), trace=False)
    return _assemble(res.results)


# revision 39
# speedup vs baseline: 1.0388x; 1.0048x over previous
"""Causal self-attention (B=2, T=2048, C=1024, H=16) on 8 TRN2 NeuronCores.

Sharding: core c handles batch b = c//4 and heads 4*(c%4) .. 4*(c%4)+3
(data-parallel over B, tensor-parallel over heads; full K/V for its heads
is computed locally from the core's QKV projection slice).

Optimizations over the f32r baseline (185us -> ~126us):
  - all matmul operands bf16 (separate LDWEIGHTS with FWL + pull-ahead
    instead of f32r's self-loading weight path; no f32r N<256 4x penalty;
    half the DMA bytes); PSUM accumulation stays f32
  - the two K=64 S-matmuls of a head pair land in disjoint PE row groups
    (auto tile_position from base partitions 0/64) and execute
    concurrently (second MM ends ~3ns after the first)
  - exp only over causally-valid columns; causal mask shrunk to the one
    128-column staircase band per diagonal tile (PV skips dead columns,
    so only the band needs zeroing) -> ~4x less gpsimd and a shorter
    exp->mask->PV critical path
  - softmax division done on the host during unshard (row 64 of each
    head's output carries the denominators); kills the on-device
    reciprocal/broadcast/multiply epilogue and its serial tail
  - startup was DMA descriptor-gen bound (~0.65us per dma_start,
    serialized per HWDGE ring): weights arrive pre-tiled so all input
    lands in 9 dma_starts issued in strict need-order, and the HAM
    warm-up bridges until the first chain's data arrives
  - projection chains are spliced between attention chunks in 4-matmul
    units so the in-order PE always has ready work while ScalarE grinds
    exp; within a batch they are grouped AFTER both K=64 S-pairs, since
    a K=128 matmul between them costs two extra 64<->128 PE array
    reconfigurations (~100-200ns each)

Per-core dataflow:
  - host passes xT = x[b].T [C,T] bf16, wqk [128, ft*1024+c*128+f] bf16
    (ft-major pre-tiled q/k weights), wv [128, c*260+f] bf16 (per head a
    65-wide block whose last column is zero)
  - qT/kT [64,T] per head via projection matmuls (contraction c on
    partitions), PSUM f32, cast to bf16 on the PSUM->SBUF copy
  - v [t,260] bf16 with a ones column appended per head (65th of a block)
  - head pairs (2p, 2p+1) share S^T tiles: s [k=128, 1024] = [S_even|S_odd]
    f32 PSUM, exp on ScalarE (scale=1/8 fused) -> bf16 pt
  - y^T [65, 512] += V'.T @ P^T accumulated over k-tiles; row 64 = softmax
    denominators (from the ones column)
  - y^T copied PSUM->SBUF bf16 (vector/scalar alternating), DMA'd out;
    host divides rows 0:64 by row 64, transposes, concats heads.
"""

import os
import sys
import types
import numpy as np
import ml_dtypes

import concourse.bass as bass
import concourse.mybir as mybir
import concourse.tile as tile
from concourse import bacc
from concourse.bass_utils import run_bass_kernel_spmd

B, T, C, H = 2, 2048, 1024, 16
D = 64
NCORES = 8
HPC = 4          # heads per core
NQB = 4          # q blocks of 512
QB = 512
F32 = mybir.dt.float32
BF16 = mybir.dt.bfloat16
EXP = mybir.ActivationFunctionType.Exp
IS_GE = mybir.AluOpType.is_ge


def _install_profhook():
    """Register the NTFF profile hook shim so BASS_TRACE=1 works; harmless
    no-op (graceful trace skip) when the axon .so lacks profiling."""
    if "antenv.axon_hooks" not in sys.modules:
        mod = types.ModuleType("antenv.axon_hooks")
        mod._hook = None
        mod.set_axon_ntff_profile_hook = lambda h: setattr(mod, "_hook", h)
        mod.get_axon_ntff_profile_hook = lambda: mod._hook
        sys.modules["antenv.axon_hooks"] = mod
        try:
            import antenv
            antenv.axon_hooks = mod
        except ImportError:
            pass
    try:
        from trn_agent_boot.trn_boot import _ntff_profile_via_ctypes
        sys.modules["antenv.axon_hooks"].set_axon_ntff_profile_hook(
            _ntff_profile_via_ctypes("/opt/axon/libaxon_pjrt.so")
        )
        import concourse.bass_utils as bu
        bu.upload_artifacts = lambda tmpdir: tmpdir
    except Exception:
        pass


_install_profhook()

_NC = None


def _build():
    nc = bacc.Bacc("TRN2", target_bir_lowering=False, debug=False,
                   num_devices=NCORES)
    # weights arrive pre-tiled on the partition axis (row p holds c-tile
    # slice [c*128+p, :] at columns c*W..) so each loads as ONE dma_start:
    # descriptor generation is ~0.65us per dma_start regardless of size,
    # and the startup was descriptor-gen bound
    xT_d = nc.declare_dram_parameter("xT", [C, T], BF16, isOutput=False)
    wqk_d = nc.declare_dram_parameter("wqk", [128, 4 * 8 * 128], BF16,
                                      isOutput=False)
    wv_d = nc.declare_dram_parameter("wv", [128, 8 * 260], BF16,
                                     isOutput=False)
    y_d = nc.declare_dram_parameter("y", [HPC, 65, T], BF16, isOutput=True)

    from contextlib import ExitStack
    with tile.TileContext(nc) as tc, ExitStack() as ctx:
        sb = ctx.enter_context(tc.tile_pool(name="sb", bufs=1))
        pp = ctx.enter_context(tc.tile_pool(name="pp", bufs=8))
        yp = ctx.enter_context(tc.tile_pool(name="yp", bufs=3))
        psp = ctx.enter_context(tc.tile_pool(name="psp", bufs=2, space="PSUM"))
        pss = ctx.enter_context(tc.tile_pool(name="pss", bufs=2, space="PSUM"))
        psy = ctx.enter_context(tc.tile_pool(name="psy", bufs=1, space="PSUM"))

        # x^T merged per t-block: xall[tb][p, c*512+f] = x[b].T[c*128+p,
        # tb*512+f] — each t-block loads as ONE dma_start (startup is both
        # descriptor-gen and bandwidth bound, so fewest gens in strict
        # need-order wins). wqk is ft-major so the first q-chain's weights
        # are their own small transfer.
        xall = [sb.tile([128, 8 * 512], BF16, name=f"xall{tb}")
                for tb in range(4)]
        wqks_all = sb.tile([128, 4 * 8 * 128], BF16, name="wqks")
        wvs_all = sb.tile([128, 8 * 260], BF16, name="wvs")
        qs = [[sb.tile([128, 512], BF16, name=f"q{p}_{tb}") for tb in range(4)]
              for p in range(2)]
        ks = [[sb.tile([128, 512], BF16, name=f"k{p}_{tb}") for tb in range(4)]
              for p in range(2)]
        vs = [sb.tile([128, 260], BF16, name=f"v_{t}") for t in range(16)]
        ones2 = sb.tile([128, 4], F32, name="ones2")
        nc.gpsimd.memset(ones2[:], 1.0)

        def xslice(tb, lo=0, hi=512):
            """AP slice of x^T covering t-block tb, contraction tile c."""
            return lambda c: xall[tb][:, c * 512 + lo:c * 512 + hi]

        # warm-up: keep the PE's HAM activity monitor busy while the input
        # DMAs land, so real matmuls start at 2.4 GHz instead of 1.2 GHz.
        # The operand memset rides the vector engine (gpsimd wakes ~3 us
        # later), and 40 matmuls (~17 cold then warm, ~6.3 us) bridge until the first
        # projection chain's inputs have arrived.
        wup = sb.tile([128, 256], BF16, name="wup")
        nc.vector.memset(wup[:], 0.5)
        wups = psp.tile([128, 256], F32, name="wups", tag="pmm")
        for _ in range(40):
            nc.tensor.matmul(wups[:], wup[:, 0:128], wup[:], start=True,
                             stop=True)

        # 8 dma_starts in strict need-order: q-pair0 weights (0.25MB), x
        # t-block 0 (1MB), k-pair0 weights, wv, pair-1 qk weights (one
        # strided transfer), then x t-blocks 1..3
        xT3 = xT_d.ap().rearrange("(c p) t -> p c t", c=8)  # [128, 8, 2048]

        def dma_xall(tb, c0=0, c1=8):
            nc.sync.dma_start(
                xall[tb][:, c0 * 512:c1 * 512].rearrange(
                    "p (c f) -> p c f", c=c1 - c0),
                xT3[:, c0:c1, tb * 512:(tb + 1) * 512])

        nc.sync.dma_start(wqks_all[:, 0:1024], wqk_d.ap()[:, 0:1024])
        # t-block 0 in halves so the first chain's c=0..3 matmuls can
        # start while c=4..7 is still in flight
        dma_xall(0, 0, 4)
        dma_xall(0, 4, 8)
        nc.sync.dma_start(wqks_all[:, 2048:3072], wqk_d.ap()[:, 2048:3072])
        nc.sync.dma_start(wvs_all[:], wv_d.ap()[:, :])
        nc.sync.dma_start(
            wqks_all[:].rearrange("p (b x) -> p b x", b=2)[:, :, 1024:2048],
            wqk_d.ap().rearrange("p (b x) -> p b x", b=2)[:, :, 1024:2048])
        for tb in (1, 2, 3):
            dma_xall(tb)

        qk_cache = {}

        def qk_chain_part(p, ft_kind, tb, half):
            """Half of one projection chain (4 accumulating matmuls); the
            second half finishes the accumulation and casts PSUM->SBUF."""
            ft = p if ft_kind == 0 else 2 + p
            key = (p, ft_kind, tb)
            if half == 0:
                qk_cache[key] = psp.tile([128, 512], F32,
                                         name=f"pqk{p}_{ft}_{tb}", tag="pmm")
            mm = qk_cache[key]
            xs = xslice(tb)
            for c in range(4 * half, 4 * half + 4):
                nc.tensor.matmul(mm[:],
                                 wqks_all[:, ft * 1024 + c * 128:
                                          ft * 1024 + (c + 1) * 128],
                                 xs(c),
                                 start=(c == 0), stop=(c == 7))
            if half == 1:
                dst = (qs if ft_kind == 0 else ks)[p][tb]
                if p == 0 and ft_kind == 1 and tb == 0:
                    # first k of the whole kernel: S(0,0) reads only
                    # cols 0:128, so give it its own small copy
                    nc.vector.tensor_copy(dst[:, 0:128], mm[:, 0:128])
                    nc.vector.tensor_copy(dst[:, 128:512], mm[:, 128:512])
                else:
                    nc.vector.tensor_copy(dst[:], mm[:])
                del qk_cache[key]

        def qk_chain(p, ft_kind, tb):
            qk_chain_part(p, ft_kind, tb, 0)
            qk_chain_part(p, ft_kind, tb, 1)

        def v_chain(tt):
            """Combined v projection for one t-tile (all 4 heads, N=260)."""
            tb, sub = tt // 4, tt % 4
            mmv = psp.tile([128, 260], F32, name=f"pv{tt}", tag="pmm")
            xs = xslice(tb, sub * 128, (sub + 1) * 128)
            for c in range(8):
                nc.tensor.matmul(mmv[:], xs(c),
                                 wvs_all[:, c * 260:(c + 1) * 260],
                                 start=(c == 0), stop=(c == 7))
            nc.vector.tensor_copy(vs[tt][:], mmv[:])
            nc.vector.tensor_copy(vs[tt][:, 64:260:65], ones2[:])

        def attn_s_part(p, j, kk, ptiles):
            """S matmuls + exp + causal mask for chunk (p, j, kk).

            Diagonal k-tiles only have valid scores for q >= k, i.e. local
            q >= off = 128*(kk-4j); matmuls and exp skip the dead columns
            (PV skips them too, so they can hold stale garbage), and only
            the 128-wide staircase band [off, off+128) needs masking."""
            off = max(0, 128 * (kk - 4 * j))
            s = pss.tile([128, 1024], F32, name=f"s{p}_{j}_{kk}", tag="s")
            ktb, ksub = kk // 4, (kk % 4) * 128
            nc.tensor.matmul(s[:, off:512],
                             ks[p][ktb][0:64, ksub:ksub + 128],
                             qs[p][j][0:64, off:512],
                             start=True, stop=True)
            nc.tensor.matmul(s[:, 512 + off:1024],
                             ks[p][ktb][64:128, ksub:ksub + 128],
                             qs[p][j][64:128, off:512],
                             start=True, stop=True)
            pt = pp.tile([128, 1024], BF16, name=f"pt{p}_{j}_{kk}", tag="pt")
            if off:
                nc.scalar.activation(
                    pt[:].rearrange("p (b q) -> p b q", b=2)[:, :, off:512],
                    s[:].rearrange("p (b q) -> p b q", b=2)[:, :, off:512],
                    EXP, scale=0.125)
            else:
                nc.scalar.activation(pt[:], s[:], EXP, scale=0.125)
            if kk >= 4 * j:
                # zero P where q < k; only the staircase band straddles the
                # diagonal (cols [off, off+128) of both head halves); the
                # predicate reduces to local_q >= key_partition (base=0)
                band = pt[:].rearrange("p (b q) -> p b q", b=2)[
                    :, :, off:off + 128]
                nc.gpsimd.affine_select(
                    band, band,
                    pattern=[[0, 2], [1, 128]],
                    compare_op=IS_GE, fill=0.0,
                    base=0,
                    channel_multiplier=-1)
            ptiles[(j, kk)] = pt

        def emit_out(p, j, state):
            """Copy the finished y^T accumulators (incl. denominator row 64)
            PSUM->SBUF as bf16 and DMA out in 4 row-chunks (spread across
            DMA queues); host does the divide. The very last output's
            second copy rides ScalarE (idle by then) so the two tail
            copies run in parallel; midstream both stay off ScalarE,
            which is the attention bottleneck (gpsimd cannot read PSUM)."""
            for h01, key in ((0, "ye"), (1, "yo")):
                ysb = yp.tile([65, 512], BF16,
                              name=f"ysb{p}_{j}_{h01}", tag="ysb")
                last = h01 == 1 and p == 1 and j == NQB - 1
                if last:
                    nc.scalar.copy(ysb[:], state[key][:])
                else:
                    nc.vector.tensor_copy(ysb[:], state[key][:])
                # one dma_start per head tile (the HW splits it across all
                # 16 SDMA engines); the final tile rides the scalar HWDGE
                # ring so the two tail DMAs drain in parallel
                eng = nc.scalar if last else nc.sync
                eng.dma_start(
                    y_d.ap()[2 * p + h01, :, j * 512:(j + 1) * 512], ysb[:])

        def attn_pv_part(p, j, kk, state, ptiles):
            """PV-accumulation closures for chunk (p, j, kk): one matmul per
            head half (so the caller can group same-PSUM-bank matmuls), plus
            a trailing output closure on the q-block's last k-tile."""
            nkt = 4 * (j + 1)
            if kk == 0:
                state["ye"] = psy.tile([65, 512], F32,
                                       name=f"ye{p}_{j}", tag="ye")
                state["yo"] = psy.tile([65, 512], F32,
                                       name=f"yo{p}_{j}", tag="yo")
            pt = ptiles.pop((j, kk))
            first, last = (kk == 0), (kk == nkt - 1)
            # skip columns where P is all-zero (above the causal diagonal);
            # their y contribution is zero and PSUM keeps the prior partials
            off = 0 if first else max(0, 128 * (kk - 4 * j))

            def mm_e():
                nc.tensor.matmul(state["ye"][:, off:512],
                                 vs[kk][:, 130 * p:130 * p + 65],
                                 pt[:, off:512],
                                 start=first, stop=last)

            def mm_o():
                nc.tensor.matmul(state["yo"][:, off:512],
                                 vs[kk][:, 130 * p + 65:130 * p + 130],
                                 pt[:, 512 + off:1024],
                                 start=first, stop=last)

            fin = (lambda: emit_out(p, j, state)) if last else None
            return mm_e, mm_o, fin

        ptiles = {}
        states = {}

        def run_pair(p, stage_work, filler=None):
            """Emit the pair's attention as one flat pipeline in batches of
            two chunks: S/exp of batch b is emitted before PV of batch b-1
            (across q-block boundaries), so the in-order PE never stalls
            behind exp. stage_work (projection chains) is emitted at a
            q-block's first chunk; filler[i] work units are spliced in
            right after chunk i's S matmuls (PE food during exp waits)."""
            seq = [(j, kk) for j in range(NQB) for kk in range(4 * (j + 1))]
            batches = [seq[i:i + 2] for i in range(0, len(seq), 2)]
            filler = filler or {}

            def emit_pv(batch):
                parts = [attn_pv_part(p, pj, pkk,
                                      states.setdefault((p, pj), {}), ptiles)
                         for (pj, pkk) in batch]
                for e, o, _ in parts:
                    e()
                    o()
                for _, _, fin in parts:
                    if fin is not None:
                        fin()

            prev = None
            ci = 0
            for batch in batches:
                # emit both chunks' K=64 S-pairs back-to-back, THEN the
                # K=128 fillers: a filler between the S-pairs would cost
                # two extra 64<->128 PE array reconfigurations (~100-200ns
                # each) per batch. Stage work (K=128) stays before the
                # S-group, adjacent to the previous batch's K=128 PVs.
                fls = []
                for (j, kk) in batch:
                    if kk == 0:
                        for w in stage_work.get(j, ()):
                            w()
                    attn_s_part(p, j, kk, ptiles)
                    fls += filler.get(ci, ())
                    ci += 1
                for w in fls:
                    w()
                if prev is not None:
                    emit_pv(prev)
                prev = batch
            emit_pv(prev)

        # pair-0 stages: its own q/k projections + the first v tile of the
        # block; the remaining v tiles and pair 1's t-block-0 projections
        # are spliced between chunks as filler (q-blocks start at chunk
        # indices 0, 4, 12, 24; v(4j+i) must land ~i chunks in, before the
        # PV that consumes it)
        stage0 = {}
        for j in range(NQB):
            stage0[j] = [lambda j=j: qk_chain(0, 0, j),
                         lambda j=j: qk_chain(0, 1, j)]
            if j:
                stage0[j].append(lambda j=j: v_chain(4 * j))
        fill0 = {0: [lambda: v_chain(0)]}
        for j, base in enumerate((0, 4, 12, 24)):
            for i in (1, 2, 3):
                fill0.setdefault(base + i - 1, []).append(
                    lambda tt=4 * j + i: v_chain(tt))

        def funit(tb, ft_kind, half):
            return lambda: qk_chain_part(1, ft_kind, tb, half)

        for ci, (tb, ft_kind, half) in [
                (27, (0, 0, 0)), (29, (0, 0, 1)),
                (31, (0, 1, 0)), (33, (0, 1, 1))]:
            fill0.setdefault(ci, []).append(funit(tb, ft_kind, half))

        # pair-1 filler: its remaining projection chains in 4-matmul units,
        # spread across the chunks of the preceding q-block (each unit must
        # land before its stage starts: stages begin at chunks 4, 12, 24)
        # due-bys (chunk of first read): q[tb] at block start (4, 12, 24);
        # k[tb] only at the diagonal, chunk offset_j + 4*tb (8, 20, 36).
        # Spread the 12 units uniformly across the exp-bound phase.
        fill1 = {}
        for ci, (tb, ft_kind, half) in [
                (0, (1, 0, 0)), (1, (1, 0, 1)), (3, (1, 1, 0)), (5, (1, 1, 1)),
                (7, (2, 0, 0)), (9, (2, 0, 1)), (13, (2, 1, 0)),
                (15, (2, 1, 1)), (18, (3, 0, 0)), (21, (3, 0, 1)),
                (27, (3, 1, 0)), (31, (3, 1, 1))]:
            fill1.setdefault(ci, []).append(funit(tb, ft_kind, half))

        run_pair(0, stage0, fill0)
        run_pair(1, {}, fill1)

    nc.compile()
    return nc


def _get_nc():
    global _NC
    if _NC is None:
        _NC = _build()
    return _NC


def _make_in_maps(x, W_attn):
    x = np.asarray(x, dtype=np.float32)
    W = np.asarray(W_attn, dtype=np.float32)
    wq, wk, wv = W[0:C], W[C:2 * C], W[2 * C:3 * C]
    bf = ml_dtypes.bfloat16
    in_maps = []
    for c in range(NCORES):
        b, g = c // 4, c % 4
        heads = [HPC * g + i for i in range(HPC)]
        xTb = np.ascontiguousarray(x[b].T).astype(bf)
        qrows = np.concatenate([wq[D * h:D * h + D] for h in heads], axis=0)
        krows = np.concatenate([wk[D * h:D * h + D] for h in heads], axis=0)
        wqk_np = np.concatenate([qrows, krows], 0).T  # [C, 512]
        wv_np = np.zeros((C, HPC * 65), np.float32)
        for i, h in enumerate(heads):
            wv_np[:, 65 * i:65 * i + D] = wv[D * h:D * h + D].T
        # pre-tile on the partition axis so each weight loads in O(1)
        # dma_starts: wqk becomes [p, ft*1024 + c*128 + f] (ft-major, so
        # the first chain's q weights are a small leading transfer), wv
        # becomes [p, c*260 + f]
        wqk_t = np.ascontiguousarray(
            wqk_np.reshape(8, 128, 4, 128).transpose(1, 2, 0, 3)
            .reshape(128, -1))
        wv_t = np.ascontiguousarray(
            wv_np.reshape(8, 128, 260).transpose(1, 0, 2).reshape(128, -1))
        in_maps.append({"xT": xTb, "wqk": wqk_t.astype(bf),
                        "wv": wv_t.astype(bf)})
    return in_maps


def _execute(in_maps, trace=False):
    return run_bass_kernel_spmd(_get_nc(), in_maps,
                                core_ids=list(range(NCORES)), trace=trace)


def _assemble(results):
    y = np.empty((B, T, C), np.float32)
    for c in range(NCORES):
        b, g = c // 4, c % 4
        # [HPC, 65, T] bf16; row 64 = softmax denominator
        yc = results[c]["y"].astype(np.float32)
        for i in range(HPC):
            h = HPC * g + i
            y[b, :, D * h:D * h + D] = (yc[i, 0:64] / yc[i, 64:65]).T
    return y


def kernel(x, W_attn):
    res = _execute(_make_in_maps(x, W_attn), trace=False)
    return _assemble(res.results)
